# revision 27
# baseline (speedup 1.0000x reference)
"""Two-layer GATv2 (DGL GATv2Conv x2 + projection) on 8 Trainium2 NeuronCores.

Sharding: nodes partitioned across 8 cores (1250 each); edges assigned to the
owner of dst; weights replicated; src features exchanged via AllGather of the
per-layer gather table (bf16).

Math: lrelu(z) = 0.6 z + 0.4 |z| (slope 0.2), so the attention logit
e = sum_d a_d lrelu(z_d) = 0.6(as_u + ad_v) + 0.4 sum_d a_d |z_d| with
as = x @ (W_s @ a), ad = x @ (W_d @ a) carried as extra table columns.
Softmax is unnormalized: numerator and denominator accumulate in the same PSUM
window via matmuls with an exp-scaled one-hot scatter matrix; division happens
per 128-node window in the epilogue.

Edge phase: dma_gather fetches up to 1024 edge rows per instruction (src rows
from the allgathered table, dst rows from the local fd table); all elementwise,
activation and reduce work is batched over whole gather blocks.

conv1 table row (640 cols): [fs_h0(256) | 1 | fs_h1(256) | 1 | a0 a1 | 0pad]
  -> per-head agg matmul rhs [fs_h | 1] is contiguous (fused denominator).
conv2 table row (1152 cols): [fs_h0(512) | fs_h1(512) | a0 a1 | 0pad]
  -> denominators via one matmul per chunk with rhs [1 | ex1/ex0].
"""
import numpy as np

N, E = 10000, 160000
IN, HID, OUT, H = 1024, 512, 512, 2
D1 = HID // H
D2 = HID
NCORES = 8
NLOC = N // NCORES
WIN = 128
NW = (NLOC + WIN - 1) // WIN
KBLK = 8

TW1 = 640
TW2 = 1152

_CACHE = {}


def _bf16(x):
    import ml_dtypes
    return np.asarray(x, dtype=np.float32).astype(ml_dtypes.bfloat16)


def _pack_idx16(flat):
    n = len(flat)
    a = np.zeros((16, n // 16), np.int16)
    a[np.arange(n) % 16, np.arange(n) // 16] = flat
    return np.tile(a, (8, 1))


def _host_prep(x, src, dst, W1s, b1s, W1d, b1d, attn1, W1r, b1r,
               W2s, b2s, W2d, b2d, attn2, W2r, b2r, Wp, bp):
    src = np.asarray(src).astype(np.int64)
    dst = np.asarray(dst).astype(np.int64)
    x = np.asarray(x, dtype=np.float32)

    core_of = dst // NLOC
    wloc = (dst % NLOC) // WIN
    e_lists = [[np.nonzero((core_of == m) & (wloc == w))[0] for w in range(NW)]
               for m in range(NCORES)]
    Mw = [max(1, max((len(e_lists[m][w]) + 127) // 128 for m in range(NCORES)))
          for w in range(NW)]
    n_chunks = int(sum(Mw))

    src_idx = np.zeros((NCORES, n_chunks * 128), np.int64)
    dst_idx = np.zeros((NCORES, n_chunks * 128), np.int64)
    # dst position within the 128-node window, per edge; 255 marks padding
    # (is_equal against column iota 0..127 then yields an all-zero row).
    dpos = np.zeros((NCORES, 128, n_chunks), np.float32)
    for m in range(NCORES):
        ci = 0
        for w in range(NW):
            el = e_lists[m][w]
            el = el[np.argsort(src[el], kind="stable")]  # DRAM row locality
            ne = len(el)
            npad = Mw[w] * 128
            s_pad = np.zeros(npad, np.int64)
            d_pad = np.zeros(npad, np.int64)
            v_pad = np.full(npad, 255, np.int64)
            sr = src[el]
            sm, srr = sr // NLOC, sr % NLOC
            sj = srr // 256
            s_pad[:ne] = sj * (NCORES * 256) + sm * 256 + (srr - sj * 256)
            d_pad[:ne] = dst[el] - m * NLOC
            v_pad[:ne] = dst[el] - m * NLOC - w * WIN
            src_idx[m, ci*128:(ci+Mw[w])*128] = s_pad
            dst_idx[m, ci*128:(ci+Mw[w])*128] = d_pad
            dpos[m, :, ci:ci+Mw[w]] = v_pad.reshape(Mw[w], 128).T
            ci += Mw[w]

    def mk_alpha(W, b, attn, d):
        ac = np.stack([W[:, h*d:(h+1)*d] @ attn[h] for h in range(H)], axis=1) * 0.6
        ab = np.array([0.6 * attn[h] @ b[h*d:(h+1)*d] for h in range(H)], np.float32)
        return ac.astype(np.float32), ab

    attn1 = np.asarray(attn1, np.float32); attn2 = np.asarray(attn2, np.float32)
    W1s = np.asarray(W1s, np.float32); W1d = np.asarray(W1d, np.float32)
    W1r = np.asarray(W1r, np.float32); W2r = np.asarray(W2r, np.float32)
    W2s = np.asarray(W2s, np.float32); W2d = np.asarray(W2d, np.float32)
    b1s = np.asarray(b1s, np.float32); b1d = np.asarray(b1d, np.float32)
    b1r = np.asarray(b1r, np.float32); b2r = np.asarray(b2r, np.float32)
    b2s = np.asarray(b2s, np.float32); b2d = np.asarray(b2d, np.float32)
    a1s, a1s_b = mk_alpha(W1s, b1s, attn1, D1)
    a1d, a1d_b = mk_alpha(W1d, b1d, attn1, D1)

    # Per-head sign-split permutation: columns with a_d >= 0 first, then
    # negative; every fs/fd table column is pre-scaled by c = 0.4|a_d| so the
    # logit reduce is a plain (pos-sum minus neg-sum) with no multiply pass.
    # conv1: h1 comes out permuted AND scaled by c; absorbed into W2* input
    # rows (host-side divide). conv2: the c-scale is divided out on-device in
    # the epilogue (fused into the existing rcp multiply), and the per-head
    # permutation rides into per-head copies of Wp.
    def sign_split(attn, d):
        perms, counts, scales = [], [], []
        for h in range(H):
            a = attn[h]
            neg = a < 0
            perm = np.argsort(neg, kind="stable")
            perms.append(perm)
            counts.append(int((~neg).sum()))
            scales.append(np.maximum(0.4 * np.abs(a[perm]), 1e-12).astype(np.float32))
        return perms, tuple(counts), scales

    perm1, P1, c1 = sign_split(attn1, D1)
    perm2, P2, c2 = sign_split(attn2, D2)

    def permscale_cols(W, perms, scales, d):
        # W [*, H*d] -> per-head column permutation and scale
        out = np.empty_like(W)
        for h in range(H):
            out[..., h*d:(h+1)*d] = W[..., h*d:(h+1)*d][..., perms[h]] * scales[h]
        return out

    W1s_t = permscale_cols(W1s, perm1, c1, D1)
    W1d_t = permscale_cols(W1d, perm1, c1, D1)
    W1r_t = permscale_cols(W1r, perm1, c1, D1)
    b1s_t = permscale_cols(b1s, perm1, c1, D1)
    b1d_t = permscale_cols(b1d, perm1, c1, D1)
    b1r_t = permscale_cols(b1r, perm1, c1, D1)

    # W2* consume h1' = c1-scaled, perm1-permuted h1: rows permuted, divided.
    def absorb_rows(W):
        out = np.empty_like(W)
        for h in range(H):
            blk = W[h*D1 + np.asarray(perm1[h])] / c1[h][:, None]
            out[h*D1:(h+1)*D1] = blk
        return out

    W2s_a = absorb_rows(W2s); W2d_a = absorb_rows(W2d); W2r_a = absorb_rows(W2r)
    # conv2 extras consume h1' (conv1-transformed), so build them from the
    # absorbed weights.
    a2s, a2s_b = mk_alpha(W2s_a, b2s, attn2, D2)
    a2d, a2d_b = mk_alpha(W2d_a, b2d, attn2, D2)
    W2s_t = permscale_cols(W2s_a, perm2, c2, D2)
    W2d_t = permscale_cols(W2d_a, perm2, c2, D2)
    b2s_t = permscale_cols(b2s, perm2, c2, D2)
    b2d_t = permscale_cols(b2d, perm2, c2, D2)
    # conv2 residual: permuted but NOT scaled (the c2 divide happens on-device
    # before the residual add).
    def perm_cols(W, perms, d):
        out = np.empty_like(W)
        for h in range(H):
            out[..., h*d:(h+1)*d] = W[..., h*d:(h+1)*d][..., perms[h]]
        return out

    W2r_t = perm_cols(W2r_a, perm2, D2)
    b2r_t = perm_cols(b2r, perm2, D2)

    # conv1 T block: [fs0 | ones | fs1 | ones | a0 a1 | pad]
    def blk1(W, alpha):
        B = np.zeros((IN, TW1), np.float32)
        B[:, 0:256] = W[:, 0:256]
        B[:, 257:513] = W[:, 256:512]
        B[:, 514:516] = alpha
        return B

    def brow1(b, ab, with_ones):
        r = np.zeros(TW1, np.float32)
        r[0:256] = b[0:256]; r[257:513] = b[256:512]; r[514:516] = ab
        if with_ones:
            r[256] = 1.0; r[513] = 1.0
        return r

    W1cat = np.concatenate([blk1(W1s_t, a1s), blk1(W1d_t, a1d), W1r_t], axis=1)
    b1cat = np.zeros((128, W1cat.shape[1]), np.float32)
    b1cat[0, 0:TW1] = brow1(b1s_t, a1s_b, True)
    b1cat[0, TW1:2*TW1] = brow1(b1d_t, a1d_b, False)
    b1cat[0, 2*TW1:] = b1r_t

    # conv2 T block: [fs0 | fs1 | a0 a1 | pad]
    def blk2(W, alpha):
        B = np.zeros((HID, TW2), np.float32)
        B[:, 0:1024] = W
        B[:, 1024:1026] = alpha
        return B

    W2cat = np.concatenate([blk2(W2s_t, a2s), blk2(W2d_t, a2d), W2r_t], axis=1)
    b2cat = np.zeros((128, W2cat.shape[1]), np.float32)
    b2cat[0, 0:1024] = b2s_t; b2cat[0, 1024:1026] = a2s_b
    b2cat[0, TW2:TW2+1024] = b2d_t; b2cat[0, TW2+1024:TW2+1026] = a2d_b
    b2cat[0, 2*TW2:] = b2r_t

    # per-head Wp with conv2's head permutation on its input rows
    Wp = np.asarray(Wp, np.float32)
    Wp2 = np.concatenate([Wp[np.asarray(perm2[h])] for h in range(H)], axis=0)

    invc2 = np.concatenate([1.0 / c2[h] for h in range(H)])
    invc2_t = np.tile(invc2.reshape(1, -1), (128, 1))

    bpcat = np.zeros((128, OUT), np.float32)
    bpcat[0, :] = np.asarray(bp, np.float32)
    has_bias = bool(max(float(np.abs(np.asarray(b, np.float32)).max()) for b in
                        (b1s, b1d, b1r, b2s, b2d, b2r, bp)) > 0)

    ident = np.eye(128, dtype=np.float32)
    ebias = np.zeros((128, 128), np.float32); ebias[0, :] = 1.0
    iotac = np.tile(np.arange(128, dtype=np.float32), (128, 1))

    shared = {
        "w1cat": _bf16(W1cat), "b1cat": _bf16(b1cat),
        "w2cat": _bf16(W2cat), "b2cat": _bf16(b2cat),
        "wp": _bf16(Wp2), "bpcat": _bf16(bpcat),
        "invc2": _bf16(invc2_t),
        "ident": _bf16(ident), "ebias": _bf16(ebias),
        "iotac": _bf16(iotac),
    }
    in_maps = []
    for m in range(NCORES):
        xm = x[m*NLOC:(m+1)*NLOC]
        xT = np.zeros((IN, 1280), np.float32)
        xT[:, :NLOC] = xm.T
        im = dict(shared)
        im["xt"] = _bf16(xT)
        im["sidx"] = _pack_idx16(src_idx[m])
        im["didx"] = _pack_idx16(dst_idx[m])
        im["dpos"] = _bf16(dpos[m])
        in_maps.append(im)
    meta = {"has_bias": has_bias, "P1": P1, "P2": P2}
    return in_maps, Mw, n_chunks, meta


def _blocks(mw):
    out, c = [], 0
    while c < mw:
        k = min(KBLK, mw - c)
        out.append((c, k))
        c += k
    return out


def _build_program(Mw, n_chunks, meta=None, repeat=1):
    has_bias = meta["has_bias"]
    P1 = meta["P1"]
    P2 = meta["P2"]
    import sys
    if "/opt/trn_rl_repo" not in sys.path:
        sys.path.insert(0, "/opt/trn_rl_repo")
    import concourse.bass as bass
    import concourse.bacc as bacc
    import concourse.mybir as mybir
    import concourse.tile as tile

    dt = mybir.dt
    AF = mybir.ActivationFunctionType
    AL = mybir.AluOpType

    nc = bacc.Bacc("TRN2", target_bir_lowering=False, debug=False,
                   num_devices=NCORES)

    W1W = 2 * TW1 + 512    # 1792
    W2W = 2 * TW2 + 1024   # 3328
    RG = [list(range(NCORES))]

    xt_d = nc.dram_tensor("xt", [IN, 1280], dt.bfloat16, kind="ExternalInput")
    w1_d = nc.dram_tensor("w1cat", [IN, W1W], dt.bfloat16, kind="ExternalInput")
    b1_d = nc.dram_tensor("b1cat", [128, W1W], dt.bfloat16, kind="ExternalInput")
    w2_d = nc.dram_tensor("w2cat", [HID, W2W], dt.bfloat16, kind="ExternalInput")
    b2_d = nc.dram_tensor("b2cat", [128, W2W], dt.bfloat16, kind="ExternalInput")
    wp_d = nc.dram_tensor("wp", [2 * HID, OUT], dt.bfloat16, kind="ExternalInput")
    bp_d = nc.dram_tensor("bpcat", [128, OUT], dt.bfloat16, kind="ExternalInput")
    ic2_d = nc.dram_tensor("invc2", [128, 1024], dt.bfloat16, kind="ExternalInput")
    id_d = nc.dram_tensor("ident", [128, 128], dt.bfloat16, kind="ExternalInput")
    eb_d = nc.dram_tensor("ebias", [128, 128], dt.bfloat16, kind="ExternalInput")
    si_d = nc.dram_tensor("sidx", [128, n_chunks * 8], dt.int16, kind="ExternalInput")
    di_d = nc.dram_tensor("didx", [128, n_chunks * 8], dt.int16, kind="ExternalInput")
    dp_d = nc.dram_tensor("dpos", [128, n_chunks], dt.bfloat16, kind="ExternalInput")
    io_d = nc.dram_tensor("iotac", [128, 128], dt.bfloat16, kind="ExternalInput")

    NCH = (NLOC + 255) // 256
    t1_own = nc.dram_tensor("t1_own", [NCH * 256, TW1], dt.bfloat16, kind="Internal")
    t1_full = nc.dram_tensor("t1_full", [NCH * NCORES * 256, TW1], dt.bfloat16,
                             kind="Internal", addr_space="Shared")
    fd1q = nc.dram_tensor("fd1q", [NLOC, 512], dt.bfloat16, kind="Internal")
    t2_own = nc.dram_tensor("t2_own", [NCH * 256, TW2], dt.bfloat16, kind="Internal")
    t2_full = nc.dram_tensor("t2_full", [NCH * NCORES * 256, TW2], dt.bfloat16,
                             kind="Internal", addr_space="Shared")
    fd2q = nc.dram_tensor("fd2q", [NLOC, 1024], dt.bfloat16, kind="Internal")
    out_d = nc.dram_tensor("out", [NLOC, OUT], dt.float32, kind="ExternalOutput")

    def mm_cols(ncols):
        splits, c = [], 0
        while c < ncols:
            n_ = min(512, ncols - c)
            splits.append((c, c + n_))
            c += n_
        return splits

    def node_matmul(ps, lhs_of_k, nk, w_sb, ww, b_sb, ones_cols, ebias):
        """Accumulate sum_k lhsT_k.T @ W_k into ps[:, 0:ww] (+ bias row)."""
        spans = mm_cols(ww)
        for k in range(nk):
            lhs = lhs_of_k(k)
            for si, (c0, c1) in enumerate(spans):
                last = (k == nk - 1)
                need_bias = has_bias or any(c0 <= oc < c1 for oc in ones_cols)
                nc.tensor.matmul(ps[:, c0:c1], lhsT=lhs, rhs=w_sb[:, k*ww+c0:k*ww+c1],
                                 start=(k == 0), stop=(last and not need_bias))
        for (c0, c1) in spans:
            need_bias = has_bias or any(c0 <= oc < c1 for oc in ones_cols)
            if need_bias:
                nc.tensor.matmul(ps[:, c0:c1], lhsT=ebias[:, :], rhs=b_sb[:, c0:c1],
                                 start=False, stop=True)

    with tile.TileContext(nc) as tc:
        with tc.tile_pool(name="cst", bufs=1) as cst, \
             tc.tile_pool(name="res", bufs=1) as resid:

            ident = cst.tile([128, 128], dt.bfloat16)
            nc.sync.dma_start(out=ident[:, :], in_=id_d[:, :])
            ebias = cst.tile([128, 128], dt.bfloat16)
            nc.sync.dma_start(out=ebias[:, :], in_=eb_d[:, :])
            ic2 = cst.tile([128, 1024], dt.bfloat16)
            nc.sync.dma_start(out=ic2[:, :], in_=ic2_d[:, :])
            sidx = cst.tile([128, n_chunks * 8], dt.int16)
            nc.sync.dma_start(out=sidx[:, :], in_=si_d[:, :])
            didx = cst.tile([128, n_chunks * 8], dt.int16)
            nc.sync.dma_start(out=didx[:, :], in_=di_d[:, :])
            dpos = cst.tile([128, n_chunks], dt.bfloat16)
            nc.sync.dma_start(out=dpos[:, :], in_=dp_d[:, :])
            iot = cst.tile([128, 128], dt.bfloat16)
            nc.sync.dma_start(out=iot[:, :], in_=io_d[:, :])

            rs1 = resid.tile([128, NW * 512], dt.bfloat16)
            h1 = resid.tile([128, NW * 512], dt.bfloat16)
            rs2 = resid.tile([128, NW * 1024], dt.bfloat16)

            for rep in range(repeat):
                # ================= conv1 node matmuls =================
                with tc.tile_pool(name="n1", bufs=2) as n1p, \
                     tc.tile_pool(name="n1w", bufs=1) as n1w, \
                     tc.tile_pool(name="n1ps", bufs=1, space="PSUM") as n1ps:
                    xt_sb = n1w.tile([128, 8 * 1280], dt.bfloat16)
                    for k in range(8):
                        nc.sync.dma_start(out=xt_sb[:, k*1280:(k+1)*1280],
                                          in_=xt_d[k*128:(k+1)*128, :])
                    w1 = n1w.tile([128, 8 * W1W], dt.bfloat16)
                    for k in range(8):
                        nc.sync.dma_start(out=w1[:, k*W1W:(k+1)*W1W],
                                          in_=w1_d[k*128:(k+1)*128, :])
                    b1 = n1w.tile([128, W1W], dt.bfloat16)
                    nc.sync.dma_start(out=b1[:, :], in_=b1_d[:, :])
                    for nt in range(NW):
                        ps = n1ps.tile([128, W1W], dt.float32, space="PSUM")
                        node_matmul(ps,
                                    lambda k: xt_sb[:, k*1280 + nt*128: k*1280 + (nt+1)*128],
                                    8, w1, W1W, b1, (256, 513), ebias)
                        t1t = n1p.tile([128, TW1], dt.bfloat16, tag="t1t")
                        nc.vector.tensor_copy(out=t1t[:, :], in_=ps[:, 0:TW1])
                        fdt = n1p.tile([128, 512], dt.bfloat16, tag="fdt")
                        nc.vector.tensor_copy(
                            out=fdt[:, :].rearrange("p (h d) -> p h d", d=256),
                            in_=ps[:, TW1:TW1+514].rearrange("p (h d) -> p h d", d=257)
                                [:, :, 0:256])
                        nc.vector.tensor_copy(out=rs1[:, nt*512:(nt+1)*512],
                                              in_=ps[:, 2*TW1:W1W])
                        rows = min(128, NLOC - nt * 128)
                        nc.sync.dma_start(out=t1_own[nt*128:nt*128+rows, :],
                                          in_=t1t[:rows, :])
                        nc.sync.dma_start(out=fd1q[nt*128:nt*128+rows, :],
                                          in_=fdt[:rows, :])
                        if nt % 2 == 1 or nt == NW - 1:
                            j = nt // 2
                            nc.gpsimd.collective_compute(
                                "AllGather", mybir.AluOpType.bypass, replica_groups=RG,
                                ins=[t1_own[j*256:(j+1)*256, :]],
                                outs=[t1_full[j*NCORES*256:(j+1)*NCORES*256, :]])

                # ================= conv1 edge phase =================
                with tc.tile_pool(name="e1a", bufs=2) as e1a, \
                     tc.tile_pool(name="e1b", bufs=2) as e1b, \
                     tc.tile_pool(name="e1agg", bufs=2, space="PSUM") as e1agg:
                    cbase = 0
                    for w in range(NW):
                        mw = Mw[w]
                        agg0 = e1agg.tile([128, 257], dt.float32, space="PSUM", tag="agg0")
                        agg1 = e1agg.tile([128, 257], dt.float32, space="PSUM", tag="agg1")
                        for bi, (c0, kb) in enumerate(_blocks(mw)):
                            ci = cbase + c0
                            first = (bi == 0)
                            last = (c0 + kb == mw)
                            gfs = e1a.tile([128, KBLK * TW1], dt.bfloat16, tag="gfs")
                            nc.gpsimd.dma_gather(
                                out_ap=gfs[:, 0:kb*TW1].rearrange("p (k t) -> p k t", t=TW1),
                                in_ap=t1_full[:, :], idxs_ap=sidx[:, ci*8:(ci+kb)*8],
                                num_idxs=kb*128, num_idxs_reg=kb*128, elem_size=TW1)
                            gfd = e1b.tile([128, KBLK * 512], dt.bfloat16, tag="gfd")
                            nc.gpsimd.dma_gather(
                                out_ap=gfd[:, 0:kb*512].rearrange("p (k t) -> p k t", t=512),
                                in_ap=fd1q[:, :], idxs_ap=didx[:, ci*8:(ci+kb)*8],
                                num_idxs=kb*128, num_idxs_reg=kb*128, elem_size=512)
                            oh = e1a.tile([128, KBLK * 128], dt.bfloat16, tag="oh")
                            nc.vector.tensor_tensor(
                                out=oh[:, 0:kb*128].rearrange("p (c o) -> p c o", o=128),
                                in0=iot[:, :].rearrange("p (u o) -> p u o", u=1)
                                    .broadcast_to([128, kb, 128]),
                                in1=dpos[:, ci:ci+kb].rearrange("p (c u) -> p c u", u=1)
                                    .broadcast_to([128, kb, 128]),
                                op=AL.is_equal)
                            gv = gfs[:, 0:kb*TW1].rearrange("p (k t) -> p k t", t=TW1)
                            z = e1b.tile([128, KBLK * 512], dt.bfloat16, tag="z")
                            nc.vector.tensor_tensor(
                                out=z[:, 0:kb*512].rearrange("p (k h d) -> p k h d", h=2, d=256),
                                in0=gv[:, :, 0:514].rearrange("p k (h d) -> p k h d", d=257)
                                    [:, :, :, 0:256],
                                in1=gfd[:, 0:kb*512].rearrange("p (k h d) -> p k h d", h=2, d=256),
                                op=AL.add)
                            nc.scalar.activation(out=z[:, 0:kb*512], in_=z[:, 0:kb*512],
                                                 func=AF.Abs)
                            zv = z[:, 0:kb*512].rearrange("p (k t) -> p k t", t=512)
                            # eab4 col layout: 4k + 2h + s (s: 0=pos, 1=neg)
                            eab = e1b.tile([128, 4 * KBLK], dt.float32, tag="eab")
                            ranges1 = ((0, 0, P1[0]), (1, P1[0], 256),
                                       (2, 256, 256 + P1[1]), (3, 256 + P1[1], 512))
                            for (g, lo, hi) in ranges1:
                                if lo == hi:
                                    nc.vector.memset(eab[:, g:4*kb:4], 0.0)
                                else:
                                    nc.vector.tensor_reduce(
                                        out=eab[:, g:4*kb:4], in_=zv[:, :, lo:hi],
                                        axis=mybir.AxisListType.X, op=AL.add)
                            # extras: only as_u — the ad_v term is constant per
                            # dst node and cancels in the edge softmax.
                            e2 = e1b.tile([128, 2 * KBLK], dt.float32, tag="e2")
                            nc.vector.tensor_tensor(
                                out=e2[:, 0:2*kb], in0=eab[:, 0:4*kb:2],
                                in1=eab[:, 1:4*kb:2], op=AL.subtract)
                            nc.vector.tensor_tensor(
                                out=e2[:, 0:2*kb].rearrange("p (k h) -> p k h", h=2),
                                in0=e2[:, 0:2*kb].rearrange("p (k h) -> p k h", h=2),
                                in1=gv[:, :, 514:516], op=AL.add)
                            ex = e1b.tile([128, 2 * KBLK], dt.float32, tag="ex")
                            nc.scalar.activation(out=ex[:, 0:2*kb], in_=e2[:, 0:2*kb],
                                                 func=AF.Exp)
                            sS = e1a.tile([128, 2 * KBLK * 128], dt.bfloat16, tag="sS")
                            for h in range(2):
                                nc.vector.tensor_tensor(
                                    out=sS[:, h*KBLK*128:h*KBLK*128+kb*128]
                                        .rearrange("p (c o) -> p c o", o=128),
                                    in0=oh[:, 0:kb*128].rearrange("p (c o) -> p c o", o=128),
                                    in1=ex[:, h:2*kb:2].rearrange("p (c u) -> p c u", u=1)
                                        .broadcast_to([128, kb, 128]),
                                    op=AL.mult)
                            for c in range(kb):
                                st = first and (c == 0)
                                sp = last and (c == kb - 1)
                                nc.tensor.matmul(
                                    agg0[:, :], lhsT=sS[:, c*128:(c+1)*128],
                                    rhs=gfs[:, c*TW1:c*TW1+257], start=st, stop=sp)
                                nc.tensor.matmul(
                                    agg1[:, :], lhsT=sS[:, KBLK*128+c*128:KBLK*128+(c+1)*128],
                                    rhs=gfs[:, c*TW1+257:c*TW1+514], start=st, stop=sp)
                        # ---- window epilogue ----
                        dsb = e1b.tile([128, 2], dt.float32, tag="dsb")
                        nc.vector.tensor_copy(out=dsb[:, 0:1], in_=agg0[:, 256:257])
                        nc.vector.tensor_copy(out=dsb[:, 1:2], in_=agg1[:, 256:257])
                        nc.vector.tensor_scalar_add(dsb[:, :], dsb[:, :], 1e-20)
                        rcp = e1b.tile([128, 2], dt.float32, tag="rcp")
                        nc.vector.reciprocal(rcp[:, :], dsb[:, :])
                        hw = h1[:, w*512:(w+1)*512]
                        nc.vector.tensor_scalar_mul(hw[:, 0:256], agg0[:, 0:256], rcp[:, 0:1])
                        nc.vector.tensor_scalar_mul(hw[:, 256:512], agg1[:, 0:256], rcp[:, 1:2])
                        nc.vector.tensor_tensor(out=hw[:, :], in0=hw[:, :],
                                                in1=rs1[:, w*512:(w+1)*512], op=AL.add)
                        nc.vector.tensor_scalar_max(hw[:, :], hw[:, :], 0.0)
                        cbase += mw

                # ================= conv2 node matmuls =================
                with tc.tile_pool(name="n2", bufs=2) as n2p, \
                     tc.tile_pool(name="n2w", bufs=1) as n2w, \
                     tc.tile_pool(name="n2ps", bufs=1, space="PSUM") as n2ps:
                    w2 = n2w.tile([128, 4 * W2W], dt.bfloat16)
                    for k in range(4):
                        nc.sync.dma_start(out=w2[:, k*W2W:(k+1)*W2W],
                                          in_=w2_d[k*128:(k+1)*128, :])
                    b2 = n2w.tile([128, W2W], dt.bfloat16)
                    nc.sync.dma_start(out=b2[:, :], in_=b2_d[:, :])
                    HALF = 1792   # bank-aligned split of 3328 (1792 + 1536)
                    for nt in range(NW):
                        tp = n2ps.tile([128, 128], dt.bfloat16, space="PSUM", tag="tp")
                        hT = n2p.tile([128, 512], dt.bfloat16, tag="hT")
                        for j in range(4):
                            nc.tensor.transpose(out=tp[:, :],
                                                in_=h1[:, nt*512+j*128:nt*512+(j+1)*128],
                                                identity=ident[:, :])
                            nc.vector.tensor_copy(out=hT[:, j*128:(j+1)*128], in_=tp[:, :])
                        t2t = n2p.tile([128, TW2], dt.bfloat16, tag="t2t")
                        fdt = n2p.tile([128, 1024], dt.bfloat16, tag="fdt2")
                        ps = n2ps.tile([128, HALF], dt.float32, space="PSUM")
                        for hf, (base, hw_) in enumerate(((0, HALF), (HALF, W2W - HALF))):
                            spans = mm_cols(hw_)
                            for k in range(4):
                                for (c0, c1) in spans:
                                    nc.tensor.matmul(
                                        ps[:, c0:c1], lhsT=hT[:, k*128:(k+1)*128],
                                        rhs=w2[:, k*W2W+base+c0:k*W2W+base+c1],
                                        start=(k == 0),
                                        stop=(k == 3 and not has_bias))
                            if has_bias:
                                for (c0, c1) in spans:
                                    nc.tensor.matmul(ps[:, c0:c1], lhsT=ebias[:, :],
                                                     rhs=b2[:, base+c0:base+c1],
                                                     start=False, stop=True)
                            seg = [(0, TW2, "t2"), (TW2, 2*TW2, "fd"), (2*TW2, W2W, "rs")]
                            for (s0, s1, kind) in seg:
                                lo, hi = max(s0, base), min(s1, base + hw_)
                                if lo >= hi:
                                    continue
                                srcv = ps[:, lo-base:hi-base]
                                if kind == "t2":
                                    nc.vector.tensor_copy(out=t2t[:, lo:hi], in_=srcv)
                                elif kind == "fd":
                                    if lo - TW2 < 1024:
                                        nc.vector.tensor_copy(
                                            out=fdt[:, lo-TW2:min(hi-TW2, 1024)],
                                            in_=srcv[:, 0:min(hi, TW2+1024)-lo])
                                else:
                                    nc.vector.tensor_copy(
                                        out=rs2[:, nt*1024+lo-2*TW2:nt*1024+hi-2*TW2],
                                        in_=srcv)
                        rows = min(128, NLOC - nt * 128)
                        nc.sync.dma_start(out=t2_own[nt*128:nt*128+rows, :],
                                          in_=t2t[:rows, :])
                        nc.sync.dma_start(out=fd2q[nt*128:nt*128+rows, :],
                                          in_=fdt[:rows, :])
                        if nt % 2 == 1 or nt == NW - 1:
                            j = nt // 2
                            nc.gpsimd.collective_compute(
                                "AllGather", mybir.AluOpType.bypass, replica_groups=RG,
                                ins=[t2_own[j*256:(j+1)*256, :]],
                                outs=[t2_full[j*NCORES*256:(j+1)*NCORES*256, :]])

                # ================= conv2 edge phase + final =================
                with tc.tile_pool(name="e2a", bufs=2) as e2a, \
                     tc.tile_pool(name="e2b", bufs=2) as e2b, \
                     tc.tile_pool(name="e2w", bufs=1) as e2w, \
                     tc.tile_pool(name="e2agg", bufs=2, space="PSUM") as e2agg, \
                     tc.tile_pool(name="e2pf", bufs=1, space="PSUM") as e2pf:
                    wp = e2w.tile([128, 8 * OUT], dt.bfloat16)
                    for k in range(8):
                        nc.sync.dma_start(out=wp[:, k*OUT:(k+1)*OUT],
                                          in_=wp_d[k*128:(k+1)*128, :])
                    bpt = e2w.tile([128, OUT], dt.bfloat16)
                    nc.sync.dma_start(out=bpt[:, :], in_=bp_d[:, :])
                    cbase = 0
                    for w in range(NW):
                        mw = Mw[w]
                        agg0 = e2agg.tile([128, 512], dt.float32, space="PSUM", tag="agg0")
                        agg1 = e2agg.tile([128, 512], dt.float32, space="PSUM", tag="agg1")
                        den = e2agg.tile([128, 2], dt.float32, space="PSUM", tag="den")
                        for bi, (c0, kb) in enumerate(_blocks(mw)):
                            ci = cbase + c0
                            first = (bi == 0)
                            last = (c0 + kb == mw)
                            gfs = e2a.tile([128, KBLK * TW2], dt.bfloat16, tag="gfs")
                            nc.gpsimd.dma_gather(
                                out_ap=gfs[:, 0:kb*TW2].rearrange("p (k t) -> p k t", t=TW2),
                                in_ap=t2_full[:, :], idxs_ap=sidx[:, ci*8:(ci+kb)*8],
                                num_idxs=kb*128, num_idxs_reg=kb*128, elem_size=TW2)
                            gfd = e2b.tile([128, KBLK * 1024], dt.bfloat16, tag="gfd")
                            nc.gpsimd.dma_gather(
                                out_ap=gfd[:, 0:kb*1024].rearrange("p (k t) -> p k t", t=1024),
                                in_ap=fd2q[:, :], idxs_ap=didx[:, ci*8:(ci+kb)*8],
                                num_idxs=kb*128, num_idxs_reg=kb*128, elem_size=1024)
                            oh = e2a.tile([128, KBLK * 128], dt.bfloat16, tag="oh")
                            nc.vector.tensor_tensor(
                                out=oh[:, 0:kb*128].rearrange("p (c o) -> p c o", o=128),
                                in0=iot[:, :].rearrange("p (u o) -> p u o", u=1)
                                    .broadcast_to([128, kb, 128]),
                                in1=dpos[:, ci:ci+kb].rearrange("p (c u) -> p c u", u=1)
                                    .broadcast_to([128, kb, 128]),
                                op=AL.is_equal)
                            gv = gfs[:, 0:kb*TW2].rearrange("p (k t) -> p k t", t=TW2)
                            z = e2b.tile([128, KBLK * 1024], dt.bfloat16, tag="z")
                            nc.vector.tensor_tensor(
                                out=z[:, 0:kb*1024].rearrange("p (k t) -> p k t", t=1024),
                                in0=gv[:, :, 0:1024],
                                in1=gfd[:, 0:kb*1024].rearrange("p (k t) -> p k t", t=1024),
                                op=AL.add)
                            nc.scalar.activation(out=z[:, 0:kb*1024], in_=z[:, 0:kb*1024],
                                                 func=AF.Abs)
                            zv = z[:, 0:kb*1024].rearrange("p (k t) -> p k t", t=1024)
                            eab = e2b.tile([128, 4 * KBLK], dt.float32, tag="eab")
                            ranges2 = ((0, 0, P2[0]), (1, P2[0], 512),
                                       (2, 512, 512 + P2[1]), (3, 512 + P2[1], 1024))
                            for (g, lo, hi) in ranges2:
                                if lo == hi:
                                    nc.vector.memset(eab[:, g:4*kb:4], 0.0)
                                else:
                                    nc.vector.tensor_reduce(
                                        out=eab[:, g:4*kb:4], in_=zv[:, :, lo:hi],
                                        axis=mybir.AxisListType.X, op=AL.add)
                            e2t = e2b.tile([128, 2 * KBLK], dt.float32, tag="e2t")
                            nc.vector.tensor_tensor(
                                out=e2t[:, 0:2*kb], in0=eab[:, 0:4*kb:2],
                                in1=eab[:, 1:4*kb:2], op=AL.subtract)
                            nc.vector.tensor_tensor(
                                out=e2t[:, 0:2*kb].rearrange("p (k h) -> p k h", h=2),
                                in0=e2t[:, 0:2*kb].rearrange("p (k h) -> p k h", h=2),
                                in1=gv[:, :, 1024:1026], op=AL.add)
                            ex = e2b.tile([128, 2 * KBLK], dt.float32, tag="ex")
                            nc.scalar.activation(out=ex[:, 0:2*kb], in_=e2t[:, 0:2*kb],
                                                 func=AF.Exp)
                            # denominator rhs: [1 | ex1/ex0] per chunk
                            dn = e2b.tile([128, 2 * KBLK], dt.bfloat16, tag="dn")
                            nc.vector.memset(dn[:, 0:2*kb], 1.0)
                            rr = e2b.tile([128, KBLK], dt.float32, tag="rr")
                            nc.vector.reciprocal(rr[:, 0:kb], ex[:, 0:2*kb:2])
                            nc.vector.tensor_tensor(out=dn[:, 1:2*kb:2], in0=rr[:, 0:kb],
                                                    in1=ex[:, 1:2*kb:2], op=AL.mult)
                            sS = e2a.tile([128, 2 * KBLK * 128], dt.bfloat16, tag="sS")
                            for h in range(2):
                                nc.vector.tensor_tensor(
                                    out=sS[:, h*KBLK*128:h*KBLK*128+kb*128]
                                        .rearrange("p (c o) -> p c o", o=128),
                                    in0=oh[:, 0:kb*128].rearrange("p (c o) -> p c o", o=128),
                                    in1=ex[:, h:2*kb:2].rearrange("p (c u) -> p c u", u=1)
                                        .broadcast_to([128, kb, 128]),
                                    op=AL.mult)
                            for c in range(kb):
                                st = first and (c == 0)
                                sp = last and (c == kb - 1)
                                nc.tensor.matmul(
                                    agg0[:, :], lhsT=sS[:, c*128:(c+1)*128],
                                    rhs=gfs[:, c*TW2:c*TW2+512], start=st, stop=sp)
                                nc.tensor.matmul(
                                    agg1[:, :], lhsT=sS[:, KBLK*128+c*128:KBLK*128+(c+1)*128],
                                    rhs=gfs[:, c*TW2+512:c*TW2+1024], start=st, stop=sp)
                                nc.tensor.matmul(
                                    den[:, :], lhsT=sS[:, c*128:(c+1)*128],
                                    rhs=dn[:, 2*c:2*c+2], start=st, stop=sp)
                        # ---- epilogue: h2 = sum_h relu(num/den + res) ----
                        dsb = e2b.tile([128, 2], dt.float32, tag="dsb")
                        nc.vector.tensor_scalar_add(dsb[:, :], den[:, :], 1e-20)
                        rcp = e2b.tile([128, 2], dt.float32, tag="rcp")
                        nc.vector.reciprocal(rcp[:, :], dsb[:, :])
                        # th_h = (agg_h * rcp_h) * invc2_h  (c2 prescale divided out)
                        th0 = e2b.tile([128, 512], dt.bfloat16, tag="th0")
                        th1 = e2b.tile([128, 512], dt.bfloat16, tag="th1")
                        nc.vector.scalar_tensor_tensor(
                            out=th0[:, :], in0=agg0[:, :], scalar=rcp[:, 0:1],
                            in1=ic2[:, 0:512], op0=AL.mult, op1=AL.mult)
                        nc.vector.scalar_tensor_tensor(
                            out=th1[:, :], in0=agg1[:, :], scalar=rcp[:, 1:2],
                            in1=ic2[:, 512:1024], op0=AL.mult, op1=AL.mult)
                        nc.vector.tensor_tensor(out=th0[:, :], in0=th0[:, :],
                                                in1=rs2[:, w*1024:w*1024+512], op=AL.add)
                        nc.vector.tensor_tensor(out=th1[:, :], in0=th1[:, :],
                                                in1=rs2[:, w*1024+512:(w+1)*1024], op=AL.add)
                        nc.vector.tensor_scalar_max(th0[:, :], th0[:, :], 0.0)
                        nc.vector.tensor_scalar_max(th1[:, :], th1[:, :], 0.0)
                        # ---- final projection: relu(sum_h relu(t_h)) == sum_h
                        # relu(t_h), so project each head separately (with its
                        # own perm2-ordered Wp copy) and accumulate in PSUM.
                        pf = e2pf.tile([128, OUT], dt.float32, space="PSUM", tag="pf")
                        h2T = e2b.tile([128, 1024], dt.bfloat16, tag="h2T")
                        for h, th in ((0, th0), (1, th1)):
                            for j in range(4):
                                jj = h * 4 + j
                                tpv = pf[:, jj*64:(jj+1)*64].bitcast(dt.bfloat16)
                                nc.tensor.transpose(out=tpv, in_=th[:, j*128:(j+1)*128],
                                                    identity=ident[:, :])
                                nc.vector.tensor_copy(out=h2T[:, jj*128:(jj+1)*128],
                                                      in_=tpv)
                        for k in range(8):
                            nc.tensor.matmul(pf[:, :], lhsT=h2T[:, k*128:(k+1)*128],
                                             rhs=wp[:, k*OUT:(k+1)*OUT],
                                             start=(k == 0), stop=(k == 7 and not has_bias))
                        if has_bias:
                            nc.tensor.matmul(pf[:, :], lhsT=ebias[:, :], rhs=bpt[:, :],
                                             start=False, stop=True)
                        of = e2b.tile([128, OUT], dt.float32, tag="of")
                        nc.vector.tensor_copy(out=of[:, :], in_=pf[:, :])
                        rows = min(128, NLOC - w * 128)
                        nc.sync.dma_start(out=out_d[w*128:w*128+rows, :],
                                          in_=of[:rows, :])
                        cbase += mw

    nc.compile()
    return nc


def kernel(**inputs) -> np.ndarray:
    import sys
    if "/opt/trn_rl_repo" not in sys.path:
        sys.path.insert(0, "/opt/trn_rl_repo")
    from concourse.bass_utils import run_bass_kernel_spmd

    in_maps, Mw, n_chunks, meta = _host_prep(**inputs)
    key = ("prog", tuple(Mw), tuple(sorted(meta.items())))
    if key not in _CACHE:
        _CACHE[key] = _build_program(Mw, n_chunks, meta)
    nc = _CACHE[key]
    res = run_bass_kernel_spmd(nc, in_maps, core_ids=list(range(NCORES)))
    return np.concatenate([res.results[m]["out"] for m in range(NCORES)], axis=0)



# revision 28
# speedup vs baseline: 1.1433x; 1.1433x over previous
"""Two-layer GATv2 (DGL GATv2Conv x2 + projection) on 8 Trainium2 NeuronCores.

Sharding: nodes partitioned across 8 cores (1250 each); edges assigned to the
owner of dst; weights replicated; src features exchanged via AllGather of the
per-layer gather table (bf16).

Math: lrelu(z) = 0.6 z + 0.4 |z| (slope 0.2), so the attention logit
e = sum_d a_d lrelu(z_d) = 0.6(as_u + ad_v) + 0.4 sum_d a_d |z_d| with
as = x @ (W_s @ a), ad = x @ (W_d @ a) carried as extra table columns.
Softmax is unnormalized: numerator and denominator accumulate in the same PSUM
window via matmuls with an exp-scaled one-hot scatter matrix; division happens
per 128-node window in the epilogue.

Edge phase: dma_gather fetches up to 1024 edge rows per instruction (src rows
from the allgathered table, dst rows from the local fd table); all elementwise,
activation and reduce work is batched over whole gather blocks.

conv1 table row (640 cols): [fs_h0(256) | 1 | fs_h1(256) | 1 | a0 a1 | 0pad]
  -> per-head agg matmul rhs [fs_h | 1] is contiguous (fused denominator).
conv2 table row (1152 cols): [fs_h0(512) | fs_h1(512) | a0 a1 | 0pad]
  -> denominators via one matmul per chunk with rhs [1 | ex1/ex0].
"""
import numpy as np

N, E = 10000, 160000
IN, HID, OUT, H = 1024, 512, 512, 2
D1 = HID // H
D2 = HID
NCORES = 8
NLOC = N // NCORES
WIN = 128
NW = (NLOC + WIN - 1) // WIN
KBLK = 8

TW1 = 640
TW2 = 1152

_CACHE = {}


def _bf16(x):
    import ml_dtypes
    return np.asarray(x, dtype=np.float32).astype(ml_dtypes.bfloat16)


def _pack_idx16(flat):
    n = len(flat)
    a = np.zeros((16, n // 16), np.int16)
    a[np.arange(n) % 16, np.arange(n) // 16] = flat
    return np.tile(a, (8, 1))


def _host_prep(x, src, dst, W1s, b1s, W1d, b1d, attn1, W1r, b1r,
               W2s, b2s, W2d, b2d, attn2, W2r, b2r, Wp, bp):
    src = np.asarray(src).astype(np.int64)
    dst = np.asarray(dst).astype(np.int64)
    x = np.asarray(x, dtype=np.float32)

    core_of = dst // NLOC
    wloc = (dst % NLOC) // WIN
    e_lists = [[np.nonzero((core_of == m) & (wloc == w))[0] for w in range(NW)]
               for m in range(NCORES)]
    Mw = [max(1, max((len(e_lists[m][w]) + 127) // 128 for m in range(NCORES)))
          for w in range(NW)]
    n_chunks = int(sum(Mw))

    src_idx = np.zeros((NCORES, n_chunks * 128), np.int64)
    dst_idx = np.zeros((NCORES, n_chunks * 128), np.int64)
    # dst position within the 128-node window, per edge; 255 marks padding
    # (is_equal against column iota 0..127 then yields an all-zero row).
    dpos = np.zeros((NCORES, 128, n_chunks), np.float32)
    for m in range(NCORES):
        ci = 0
        for w in range(NW):
            el = e_lists[m][w]
            el = el[np.argsort(src[el], kind="stable")]  # DRAM row locality
            ne = len(el)
            npad = Mw[w] * 128
            s_pad = np.zeros(npad, np.int64)
            d_pad = np.zeros(npad, np.int64)
            v_pad = np.full(npad, 255, np.int64)
            sr = src[el]
            sm, srr = sr // NLOC, sr % NLOC
            sj = srr // 640
            s_pad[:ne] = sj * (NCORES * 640) + sm * 640 + (srr - sj * 640)
            d_pad[:ne] = dst[el] - m * NLOC
            v_pad[:ne] = dst[el] - m * NLOC - w * WIN
            src_idx[m, ci*128:(ci+Mw[w])*128] = s_pad
            dst_idx[m, ci*128:(ci+Mw[w])*128] = d_pad
            dpos[m, :, ci:ci+Mw[w]] = v_pad.reshape(Mw[w], 128).T
            ci += Mw[w]

    def mk_alpha(W, b, attn, d):
        ac = np.stack([W[:, h*d:(h+1)*d] @ attn[h] for h in range(H)], axis=1) * 0.6
        ab = np.array([0.6 * attn[h] @ b[h*d:(h+1)*d] for h in range(H)], np.float32)
        return ac.astype(np.float32), ab

    attn1 = np.asarray(attn1, np.float32); attn2 = np.asarray(attn2, np.float32)
    W1s = np.asarray(W1s, np.float32); W1d = np.asarray(W1d, np.float32)
    W1r = np.asarray(W1r, np.float32); W2r = np.asarray(W2r, np.float32)
    W2s = np.asarray(W2s, np.float32); W2d = np.asarray(W2d, np.float32)
    b1s = np.asarray(b1s, np.float32); b1d = np.asarray(b1d, np.float32)
    b1r = np.asarray(b1r, np.float32); b2r = np.asarray(b2r, np.float32)
    b2s = np.asarray(b2s, np.float32); b2d = np.asarray(b2d, np.float32)
    a1s, a1s_b = mk_alpha(W1s, b1s, attn1, D1)
    a1d, a1d_b = mk_alpha(W1d, b1d, attn1, D1)

    # Per-head sign-split permutation: columns with a_d >= 0 first, then
    # negative; every fs/fd table column is pre-scaled by c = 0.4|a_d| so the
    # logit reduce is a plain (pos-sum minus neg-sum) with no multiply pass.
    # conv1: h1 comes out permuted AND scaled by c; absorbed into W2* input
    # rows (host-side divide). conv2: the c-scale is divided out on-device in
    # the epilogue (fused into the existing rcp multiply), and the per-head
    # permutation rides into per-head copies of Wp.
    def sign_split(attn, d):
        perms, counts, scales = [], [], []
        for h in range(H):
            a = attn[h]
            neg = a < 0
            perm = np.argsort(neg, kind="stable")
            perms.append(perm)
            counts.append(int((~neg).sum()))
            scales.append(np.maximum(0.4 * np.abs(a[perm]), 1e-12).astype(np.float32))
        return perms, tuple(counts), scales

    perm1, P1, c1 = sign_split(attn1, D1)
    perm2, P2, c2 = sign_split(attn2, D2)

    def permscale_cols(W, perms, scales, d):
        # W [*, H*d] -> per-head column permutation and scale
        out = np.empty_like(W)
        for h in range(H):
            out[..., h*d:(h+1)*d] = W[..., h*d:(h+1)*d][..., perms[h]] * scales[h]
        return out

    W1s_t = permscale_cols(W1s, perm1, c1, D1)
    W1d_t = permscale_cols(W1d, perm1, c1, D1)
    W1r_t = permscale_cols(W1r, perm1, c1, D1)
    b1s_t = permscale_cols(b1s, perm1, c1, D1)
    b1d_t = permscale_cols(b1d, perm1, c1, D1)
    b1r_t = permscale_cols(b1r, perm1, c1, D1)

    # W2* consume h1' = c1-scaled, perm1-permuted h1: rows permuted, divided.
    def absorb_rows(W):
        out = np.empty_like(W)
        for h in range(H):
            blk = W[h*D1 + np.asarray(perm1[h])] / c1[h][:, None]
            out[h*D1:(h+1)*D1] = blk
        return out

    W2s_a = absorb_rows(W2s); W2d_a = absorb_rows(W2d); W2r_a = absorb_rows(W2r)
    # conv2 extras consume h1' (conv1-transformed), so build them from the
    # absorbed weights.
    a2s, a2s_b = mk_alpha(W2s_a, b2s, attn2, D2)
    a2d, a2d_b = mk_alpha(W2d_a, b2d, attn2, D2)
    W2s_t = permscale_cols(W2s_a, perm2, c2, D2)
    W2d_t = permscale_cols(W2d_a, perm2, c2, D2)
    b2s_t = permscale_cols(b2s, perm2, c2, D2)
    b2d_t = permscale_cols(b2d, perm2, c2, D2)
    # conv2 residual: permuted but NOT scaled (the c2 divide happens on-device
    # before the residual add).
    def perm_cols(W, perms, d):
        out = np.empty_like(W)
        for h in range(H):
            out[..., h*d:(h+1)*d] = W[..., h*d:(h+1)*d][..., perms[h]]
        return out

    W2r_t = perm_cols(W2r_a, perm2, D2)
    b2r_t = perm_cols(b2r, perm2, D2)

    # conv1 T block: [fs0 | ones | fs1 | ones | a0 a1 | pad]
    def blk1(W, alpha):
        B = np.zeros((IN, TW1), np.float32)
        B[:, 0:256] = W[:, 0:256]
        B[:, 257:513] = W[:, 256:512]
        B[:, 514:516] = alpha
        return B

    def brow1(b, ab, with_ones):
        r = np.zeros(TW1, np.float32)
        r[0:256] = b[0:256]; r[257:513] = b[256:512]; r[514:516] = ab
        if with_ones:
            r[256] = 1.0; r[513] = 1.0
        return r

    W1cat = np.concatenate([blk1(W1s_t, a1s), blk1(W1d_t, a1d), W1r_t], axis=1)
    b1cat = np.zeros((128, W1cat.shape[1]), np.float32)
    b1cat[0, 0:TW1] = brow1(b1s_t, a1s_b, True)
    b1cat[0, TW1:2*TW1] = brow1(b1d_t, a1d_b, False)
    b1cat[0, 2*TW1:] = b1r_t

    # conv2 T block: [fs0 | fs1 | a0 a1 | pad]
    def blk2(W, alpha):
        B = np.zeros((HID, TW2), np.float32)
        B[:, 0:1024] = W
        B[:, 1024:1026] = alpha
        return B

    W2cat = np.concatenate([blk2(W2s_t, a2s), blk2(W2d_t, a2d), W2r_t], axis=1)
    b2cat = np.zeros((128, W2cat.shape[1]), np.float32)
    b2cat[0, 0:1024] = b2s_t; b2cat[0, 1024:1026] = a2s_b
    b2cat[0, TW2:TW2+1024] = b2d_t; b2cat[0, TW2+1024:TW2+1026] = a2d_b
    b2cat[0, 2*TW2:] = b2r_t

    # per-head Wp with conv2's head permutation on its input rows
    Wp = np.asarray(Wp, np.float32)
    Wp2 = np.concatenate([Wp[np.asarray(perm2[h])] for h in range(H)], axis=0)

    invc2 = np.concatenate([1.0 / c2[h] for h in range(H)])
    invc2_t = np.tile(invc2.reshape(1, -1), (128, 1))

    bpcat = np.zeros((128, OUT), np.float32)
    bpcat[0, :] = np.asarray(bp, np.float32)
    has_bias = bool(max(float(np.abs(np.asarray(b, np.float32)).max()) for b in
                        (b1s, b1d, b1r, b2s, b2d, b2r, bp)) > 0)

    ident = np.eye(128, dtype=np.float32)
    ebias = np.zeros((128, 128), np.float32); ebias[0, :] = 1.0
    iotac = np.tile(np.arange(128, dtype=np.float32), (128, 1))

    shared = {
        "w1cat": _bf16(W1cat), "b1cat": _bf16(b1cat),
        "w2cat": _bf16(W2cat), "b2cat": _bf16(b2cat),
        "wp": _bf16(Wp2), "bpcat": _bf16(bpcat),
        "invc2": _bf16(invc2_t),
        "ident": _bf16(ident), "ebias": _bf16(ebias),
        "iotac": _bf16(iotac),
    }
    in_maps = []
    for m in range(NCORES):
        xm = x[m*NLOC:(m+1)*NLOC]
        xT = np.zeros((IN, 1280), np.float32)
        xT[:, :NLOC] = xm.T
        im = dict(shared)
        im["xt"] = _bf16(xT)
        im["sidx"] = _pack_idx16(src_idx[m])
        im["didx"] = _pack_idx16(dst_idx[m])
        im["dpos"] = _bf16(dpos[m])
        in_maps.append(im)
    meta = {"has_bias": has_bias, "P1": P1, "P2": P2}
    return in_maps, Mw, n_chunks, meta


def _blocks(mw):
    out, c = [], 0
    while c < mw:
        k = min(KBLK, mw - c)
        out.append((c, k))
        c += k
    return out


def _build_program(Mw, n_chunks, meta=None, repeat=1):
    has_bias = meta["has_bias"]
    P1 = meta["P1"]
    P2 = meta["P2"]
    import sys
    if "/opt/trn_rl_repo" not in sys.path:
        sys.path.insert(0, "/opt/trn_rl_repo")
    import concourse.bass as bass
    import concourse.bacc as bacc
    import concourse.mybir as mybir
    import concourse.tile as tile

    dt = mybir.dt
    AF = mybir.ActivationFunctionType
    AL = mybir.AluOpType

    nc = bacc.Bacc("TRN2", target_bir_lowering=False, debug=False,
                   num_devices=NCORES)

    W1W = 2 * TW1 + 512    # 1792
    W2W = 2 * TW2 + 1024   # 3328
    RG = [list(range(NCORES))]

    xt_d = nc.dram_tensor("xt", [IN, 1280], dt.bfloat16, kind="ExternalInput")
    w1_d = nc.dram_tensor("w1cat", [IN, W1W], dt.bfloat16, kind="ExternalInput")
    b1_d = nc.dram_tensor("b1cat", [128, W1W], dt.bfloat16, kind="ExternalInput")
    w2_d = nc.dram_tensor("w2cat", [HID, W2W], dt.bfloat16, kind="ExternalInput")
    b2_d = nc.dram_tensor("b2cat", [128, W2W], dt.bfloat16, kind="ExternalInput")
    wp_d = nc.dram_tensor("wp", [2 * HID, OUT], dt.bfloat16, kind="ExternalInput")
    bp_d = nc.dram_tensor("bpcat", [128, OUT], dt.bfloat16, kind="ExternalInput")
    ic2_d = nc.dram_tensor("invc2", [128, 1024], dt.bfloat16, kind="ExternalInput")
    id_d = nc.dram_tensor("ident", [128, 128], dt.bfloat16, kind="ExternalInput")
    eb_d = nc.dram_tensor("ebias", [128, 128], dt.bfloat16, kind="ExternalInput")
    si_d = nc.dram_tensor("sidx", [128, n_chunks * 8], dt.int16, kind="ExternalInput")
    di_d = nc.dram_tensor("didx", [128, n_chunks * 8], dt.int16, kind="ExternalInput")
    dp_d = nc.dram_tensor("dpos", [128, n_chunks], dt.bfloat16, kind="ExternalInput")
    io_d = nc.dram_tensor("iotac", [128, 128], dt.bfloat16, kind="ExternalInput")

    NCH = (NLOC + 639) // 640
    t1_own = nc.dram_tensor("t1_own", [NCH * 640, TW1], dt.bfloat16, kind="Internal")
    t1_full = nc.dram_tensor("t1_full", [NCH * NCORES * 640, TW1], dt.bfloat16,
                             kind="Internal", addr_space="Shared")
    fd1q = nc.dram_tensor("fd1q", [NLOC, 512], dt.bfloat16, kind="Internal")
    t2_own = nc.dram_tensor("t2_own", [NCH * 640, TW2], dt.bfloat16, kind="Internal")
    t2_full = nc.dram_tensor("t2_full", [NCH * NCORES * 640, TW2], dt.bfloat16,
                             kind="Internal", addr_space="Shared")
    fd2q = nc.dram_tensor("fd2q", [NLOC, 1024], dt.bfloat16, kind="Internal")
    out_d = nc.dram_tensor("out", [NLOC, OUT], dt.float32, kind="ExternalOutput")

    def mm_cols(ncols):
        splits, c = [], 0
        while c < ncols:
            n_ = min(512, ncols - c)
            splits.append((c, c + n_))
            c += n_
        return splits

    def node_matmul(ps, lhs_of_k, nk, w_sb, ww, b_sb, ones_cols, ebias):
        """Accumulate sum_k lhsT_k.T @ W_k into ps[:, 0:ww] (+ bias row)."""
        spans = mm_cols(ww)
        for k in range(nk):
            lhs = lhs_of_k(k)
            for si, (c0, c1) in enumerate(spans):
                last = (k == nk - 1)
                need_bias = has_bias or any(c0 <= oc < c1 for oc in ones_cols)
                nc.tensor.matmul(ps[:, c0:c1], lhsT=lhs, rhs=w_sb[:, k*ww+c0:k*ww+c1],
                                 start=(k == 0), stop=(last and not need_bias))
        for (c0, c1) in spans:
            need_bias = has_bias or any(c0 <= oc < c1 for oc in ones_cols)
            if need_bias:
                nc.tensor.matmul(ps[:, c0:c1], lhsT=ebias[:, :], rhs=b_sb[:, c0:c1],
                                 start=False, stop=True)

    with tile.TileContext(nc) as tc:
        with tc.tile_pool(name="cst", bufs=1) as cst, \
             tc.tile_pool(name="res", bufs=1) as resid:

            ident = cst.tile([128, 128], dt.bfloat16)
            nc.sync.dma_start(out=ident[:, :], in_=id_d[:, :])
            ebias = cst.tile([128, 128], dt.bfloat16)
            nc.sync.dma_start(out=ebias[:, :], in_=eb_d[:, :])
            ic2 = cst.tile([128, 1024], dt.bfloat16)
            nc.sync.dma_start(out=ic2[:, :], in_=ic2_d[:, :])
            sidx = cst.tile([128, n_chunks * 8], dt.int16)
            nc.sync.dma_start(out=sidx[:, :], in_=si_d[:, :])
            didx = cst.tile([128, n_chunks * 8], dt.int16)
            nc.sync.dma_start(out=didx[:, :], in_=di_d[:, :])
            dpos = cst.tile([128, n_chunks], dt.bfloat16)
            nc.sync.dma_start(out=dpos[:, :], in_=dp_d[:, :])
            iot = cst.tile([128, 128], dt.bfloat16)
            nc.sync.dma_start(out=iot[:, :], in_=io_d[:, :])

            rs1 = resid.tile([128, NW * 512], dt.bfloat16)
            h1 = resid.tile([128, NW * 512], dt.bfloat16)
            rs2 = resid.tile([128, NW * 1024], dt.bfloat16)

            for rep in range(repeat):
                # ================= conv1 node matmuls =================
                with tc.tile_pool(name="n1", bufs=2) as n1p, \
                     tc.tile_pool(name="n1w", bufs=1) as n1w, \
                     tc.tile_pool(name="n1ps", bufs=1, space="PSUM") as n1ps:
                    xt_sb = n1w.tile([128, 8 * 1280], dt.bfloat16)
                    for k in range(8):
                        nc.sync.dma_start(out=xt_sb[:, k*1280:(k+1)*1280],
                                          in_=xt_d[k*128:(k+1)*128, :])
                    w1 = n1w.tile([128, 8 * W1W], dt.bfloat16)
                    for k in range(8):
                        nc.sync.dma_start(out=w1[:, k*W1W:(k+1)*W1W],
                                          in_=w1_d[k*128:(k+1)*128, :])
                    b1 = n1w.tile([128, W1W], dt.bfloat16)
                    nc.sync.dma_start(out=b1[:, :], in_=b1_d[:, :])
                    for nt in range(NW):
                        ps = n1ps.tile([128, W1W], dt.float32, space="PSUM")
                        node_matmul(ps,
                                    lambda k: xt_sb[:, k*1280 + nt*128: k*1280 + (nt+1)*128],
                                    8, w1, W1W, b1, (256, 513), ebias)
                        t1t = n1p.tile([128, TW1], dt.bfloat16, tag="t1t")
                        nc.vector.tensor_copy(out=t1t[:, :], in_=ps[:, 0:TW1])
                        fdt = n1p.tile([128, 512], dt.bfloat16, tag="fdt")
                        nc.vector.tensor_copy(
                            out=fdt[:, :].rearrange("p (h d) -> p h d", d=256),
                            in_=ps[:, TW1:TW1+514].rearrange("p (h d) -> p h d", d=257)
                                [:, :, 0:256])
                        nc.vector.tensor_copy(out=rs1[:, nt*512:(nt+1)*512],
                                              in_=ps[:, 2*TW1:W1W])
                        rows = min(128, NLOC - nt * 128)
                        nc.sync.dma_start(out=t1_own[nt*128:nt*128+rows, :],
                                          in_=t1t[:rows, :])
                        nc.sync.dma_start(out=fd1q[nt*128:nt*128+rows, :],
                                          in_=fdt[:rows, :])
                        if (nt + 1) * 128 % 640 == 0 or nt == NW - 1:
                            j = nt // 5
                            nc.gpsimd.collective_compute(
                                "AllGather", mybir.AluOpType.bypass, replica_groups=RG,
                                ins=[t1_own[j*640:(j+1)*640, :]],
                                outs=[t1_full[j*NCORES*640:(j+1)*NCORES*640, :]])

                # ================= conv1 edge phase =================
                with tc.tile_pool(name="e1a", bufs=2) as e1a, \
                     tc.tile_pool(name="e1b", bufs=2) as e1b, \
                     tc.tile_pool(name="e1agg", bufs=2, space="PSUM") as e1agg:
                    cbase = 0
                    for w in range(NW):
                        mw = Mw[w]
                        agg0 = e1agg.tile([128, 257], dt.float32, space="PSUM", tag="agg0")
                        agg1 = e1agg.tile([128, 257], dt.float32, space="PSUM", tag="agg1")
                        for bi, (c0, kb) in enumerate(_blocks(mw)):
                            ci = cbase + c0
                            first = (bi == 0)
                            last = (c0 + kb == mw)
                            gfs = e1a.tile([128, KBLK * TW1], dt.bfloat16, tag="gfs")
                            nc.gpsimd.dma_gather(
                                out_ap=gfs[:, 0:kb*TW1].rearrange("p (k t) -> p k t", t=TW1),
                                in_ap=t1_full[:, :], idxs_ap=sidx[:, ci*8:(ci+kb)*8],
                                num_idxs=kb*128, num_idxs_reg=kb*128, elem_size=TW1)
                            gfd = e1b.tile([128, KBLK * 512], dt.bfloat16, tag="gfd")
                            nc.gpsimd.dma_gather(
                                out_ap=gfd[:, 0:kb*512].rearrange("p (k t) -> p k t", t=512),
                                in_ap=fd1q[:, :], idxs_ap=didx[:, ci*8:(ci+kb)*8],
                                num_idxs=kb*128, num_idxs_reg=kb*128, elem_size=512)
                            oh = e1a.tile([128, KBLK * 128], dt.bfloat16, tag="oh")
                            nc.vector.tensor_tensor(
                                out=oh[:, 0:kb*128].rearrange("p (c o) -> p c o", o=128),
                                in0=iot[:, :].rearrange("p (u o) -> p u o", u=1)
                                    .broadcast_to([128, kb, 128]),
                                in1=dpos[:, ci:ci+kb].rearrange("p (c u) -> p c u", u=1)
                                    .broadcast_to([128, kb, 128]),
                                op=AL.is_equal)
                            gv = gfs[:, 0:kb*TW1].rearrange("p (k t) -> p k t", t=TW1)
                            z = e1b.tile([128, KBLK * 512], dt.bfloat16, tag="z")
                            nc.vector.tensor_tensor(
                                out=z[:, 0:kb*512].rearrange("p (k h d) -> p k h d", h=2, d=256),
                                in0=gv[:, :, 0:514].rearrange("p k (h d) -> p k h d", d=257)
                                    [:, :, :, 0:256],
                                in1=gfd[:, 0:kb*512].rearrange("p (k h d) -> p k h d", h=2, d=256),
                                op=AL.add)
                            nc.scalar.activation(out=z[:, 0:kb*512], in_=z[:, 0:kb*512],
                                                 func=AF.Abs)
                            zv = z[:, 0:kb*512].rearrange("p (k t) -> p k t", t=512)
                            # eab4 col layout: 4k + 2h + s (s: 0=pos, 1=neg)
                            eab = e1b.tile([128, 4 * KBLK], dt.float32, tag="eab")
                            ranges1 = ((0, 0, P1[0]), (1, P1[0], 256),
                                       (2, 256, 256 + P1[1]), (3, 256 + P1[1], 512))
                            for (g, lo, hi) in ranges1:
                                if lo == hi:
                                    nc.vector.memset(eab[:, g:4*kb:4], 0.0)
                                else:
                                    nc.vector.tensor_reduce(
                                        out=eab[:, g:4*kb:4], in_=zv[:, :, lo:hi],
                                        axis=mybir.AxisListType.X, op=AL.add)
                            # extras: only as_u — the ad_v term is constant per
                            # dst node and cancels in the edge softmax.
                            e2 = e1b.tile([128, 2 * KBLK], dt.float32, tag="e2")
                            nc.vector.tensor_tensor(
                                out=e2[:, 0:2*kb], in0=eab[:, 0:4*kb:2],
                                in1=eab[:, 1:4*kb:2], op=AL.subtract)
                            nc.vector.tensor_tensor(
                                out=e2[:, 0:2*kb].rearrange("p (k h) -> p k h", h=2),
                                in0=e2[:, 0:2*kb].rearrange("p (k h) -> p k h", h=2),
                                in1=gv[:, :, 514:516], op=AL.add)
                            ex = e1b.tile([128, 2 * KBLK], dt.float32, tag="ex")
                            nc.scalar.activation(out=ex[:, 0:2*kb], in_=e2[:, 0:2*kb],
                                                 func=AF.Exp)
                            sS = e1a.tile([128, 2 * KBLK * 128], dt.bfloat16, tag="sS")
                            for h in range(2):
                                nc.vector.tensor_tensor(
                                    out=sS[:, h*KBLK*128:h*KBLK*128+kb*128]
                                        .rearrange("p (c o) -> p c o", o=128),
                                    in0=oh[:, 0:kb*128].rearrange("p (c o) -> p c o", o=128),
                                    in1=ex[:, h:2*kb:2].rearrange("p (c u) -> p c u", u=1)
                                        .broadcast_to([128, kb, 128]),
                                    op=AL.mult)
                            for c in range(kb):
                                st = first and (c == 0)
                                sp = last and (c == kb - 1)
                                nc.tensor.matmul(
                                    agg0[:, :], lhsT=sS[:, c*128:(c+1)*128],
                                    rhs=gfs[:, c*TW1:c*TW1+257], start=st, stop=sp)
                                nc.tensor.matmul(
                                    agg1[:, :], lhsT=sS[:, KBLK*128+c*128:KBLK*128+(c+1)*128],
                                    rhs=gfs[:, c*TW1+257:c*TW1+514], start=st, stop=sp)
                        # ---- window epilogue ----
                        dsb = e1b.tile([128, 2], dt.float32, tag="dsb")
                        nc.vector.tensor_copy(out=dsb[:, 0:1], in_=agg0[:, 256:257])
                        nc.vector.tensor_copy(out=dsb[:, 1:2], in_=agg1[:, 256:257])
                        nc.vector.tensor_scalar_add(dsb[:, :], dsb[:, :], 1e-20)
                        rcp = e1b.tile([128, 2], dt.float32, tag="rcp")
                        nc.vector.reciprocal(rcp[:, :], dsb[:, :])
                        hw = h1[:, w*512:(w+1)*512]
                        nc.vector.tensor_scalar_mul(hw[:, 0:256], agg0[:, 0:256], rcp[:, 0:1])
                        nc.vector.tensor_scalar_mul(hw[:, 256:512], agg1[:, 0:256], rcp[:, 1:2])
                        nc.vector.tensor_tensor(out=hw[:, :], in0=hw[:, :],
                                                in1=rs1[:, w*512:(w+1)*512], op=AL.add)
                        nc.vector.tensor_scalar_max(hw[:, :], hw[:, :], 0.0)
                        cbase += mw

                # ================= conv2 node matmuls =================
                with tc.tile_pool(name="n2", bufs=2) as n2p, \
                     tc.tile_pool(name="n2w", bufs=1) as n2w, \
                     tc.tile_pool(name="n2ps", bufs=1, space="PSUM") as n2ps:
                    w2 = n2w.tile([128, 4 * W2W], dt.bfloat16)
                    for k in range(4):
                        nc.sync.dma_start(out=w2[:, k*W2W:(k+1)*W2W],
                                          in_=w2_d[k*128:(k+1)*128, :])
                    b2 = n2w.tile([128, W2W], dt.bfloat16)
                    nc.sync.dma_start(out=b2[:, :], in_=b2_d[:, :])
                    HALF = 1792   # bank-aligned split of 3328 (1792 + 1536)
                    for nt in range(NW):
                        tp = n2ps.tile([128, 128], dt.bfloat16, space="PSUM", tag="tp")
                        hT = n2p.tile([128, 512], dt.bfloat16, tag="hT")
                        for j in range(4):
                            nc.tensor.transpose(out=tp[:, :],
                                                in_=h1[:, nt*512+j*128:nt*512+(j+1)*128],
                                                identity=ident[:, :])
                            nc.vector.tensor_copy(out=hT[:, j*128:(j+1)*128], in_=tp[:, :])
                        t2t = n2p.tile([128, TW2], dt.bfloat16, tag="t2t")
                        fdt = n2p.tile([128, 1024], dt.bfloat16, tag="fdt2")
                        ps = n2ps.tile([128, HALF], dt.float32, space="PSUM")
                        for hf, (base, hw_) in enumerate(((0, HALF), (HALF, W2W - HALF))):
                            spans = mm_cols(hw_)
                            for k in range(4):
                                for (c0, c1) in spans:
                                    nc.tensor.matmul(
                                        ps[:, c0:c1], lhsT=hT[:, k*128:(k+1)*128],
                                        rhs=w2[:, k*W2W+base+c0:k*W2W+base+c1],
                                        start=(k == 0),
                                        stop=(k == 3 and not has_bias))
                            if has_bias:
                                for (c0, c1) in spans:
                                    nc.tensor.matmul(ps[:, c0:c1], lhsT=ebias[:, :],
                                                     rhs=b2[:, base+c0:base+c1],
                                                     start=False, stop=True)
                            seg = [(0, TW2, "t2"), (TW2, 2*TW2, "fd"), (2*TW2, W2W, "rs")]
                            for (s0, s1, kind) in seg:
                                lo, hi = max(s0, base), min(s1, base + hw_)
                                if lo >= hi:
                                    continue
                                srcv = ps[:, lo-base:hi-base]
                                if kind == "t2":
                                    nc.vector.tensor_copy(out=t2t[:, lo:hi], in_=srcv)
                                elif kind == "fd":
                                    if lo - TW2 < 1024:
                                        nc.vector.tensor_copy(
                                            out=fdt[:, lo-TW2:min(hi-TW2, 1024)],
                                            in_=srcv[:, 0:min(hi, TW2+1024)-lo])
                                else:
                                    nc.vector.tensor_copy(
                                        out=rs2[:, nt*1024+lo-2*TW2:nt*1024+hi-2*TW2],
                                        in_=srcv)
                        rows = min(128, NLOC - nt * 128)
                        nc.sync.dma_start(out=t2_own[nt*128:nt*128+rows, :],
                                          in_=t2t[:rows, :])
                        nc.sync.dma_start(out=fd2q[nt*128:nt*128+rows, :],
                                          in_=fdt[:rows, :])
                        if (nt + 1) * 128 % 640 == 0 or nt == NW - 1:
                            j = nt // 5
                            nc.gpsimd.collective_compute(
                                "AllGather", mybir.AluOpType.bypass, replica_groups=RG,
                                ins=[t2_own[j*640:(j+1)*640, :]],
                                outs=[t2_full[j*NCORES*640:(j+1)*NCORES*640, :]])

                # ================= conv2 edge phase + final =================
                with tc.tile_pool(name="e2a", bufs=2) as e2a, \
                     tc.tile_pool(name="e2b", bufs=2) as e2b, \
                     tc.tile_pool(name="e2w", bufs=1) as e2w, \
                     tc.tile_pool(name="e2agg", bufs=2, space="PSUM") as e2agg, \
                     tc.tile_pool(name="e2pf", bufs=1, space="PSUM") as e2pf:
                    wp = e2w.tile([128, 8 * OUT], dt.bfloat16)
                    for k in range(8):
                        nc.sync.dma_start(out=wp[:, k*OUT:(k+1)*OUT],
                                          in_=wp_d[k*128:(k+1)*128, :])
                    bpt = e2w.tile([128, OUT], dt.bfloat16)
                    nc.sync.dma_start(out=bpt[:, :], in_=bp_d[:, :])
                    cbase = 0
                    for w in range(NW):
                        mw = Mw[w]
                        agg0 = e2agg.tile([128, 512], dt.float32, space="PSUM", tag="agg0")
                        agg1 = e2agg.tile([128, 512], dt.float32, space="PSUM", tag="agg1")
                        den = e2agg.tile([128, 2], dt.float32, space="PSUM", tag="den")
                        for bi, (c0, kb) in enumerate(_blocks(mw)):
                            ci = cbase + c0
                            first = (bi == 0)
                            last = (c0 + kb == mw)
                            gfs = e2a.tile([128, KBLK * TW2], dt.bfloat16, tag="gfs")
                            nc.gpsimd.dma_gather(
                                out_ap=gfs[:, 0:kb*TW2].rearrange("p (k t) -> p k t", t=TW2),
                                in_ap=t2_full[:, :], idxs_ap=sidx[:, ci*8:(ci+kb)*8],
                                num_idxs=kb*128, num_idxs_reg=kb*128, elem_size=TW2)
                            gfd = e2b.tile([128, KBLK * 1024], dt.bfloat16, tag="gfd")
                            nc.gpsimd.dma_gather(
                                out_ap=gfd[:, 0:kb*1024].rearrange("p (k t) -> p k t", t=1024),
                                in_ap=fd2q[:, :], idxs_ap=didx[:, ci*8:(ci+kb)*8],
                                num_idxs=kb*128, num_idxs_reg=kb*128, elem_size=1024)
                            oh = e2a.tile([128, KBLK * 128], dt.bfloat16, tag="oh")
                            nc.vector.tensor_tensor(
                                out=oh[:, 0:kb*128].rearrange("p (c o) -> p c o", o=128),
                                in0=iot[:, :].rearrange("p (u o) -> p u o", u=1)
                                    .broadcast_to([128, kb, 128]),
                                in1=dpos[:, ci:ci+kb].rearrange("p (c u) -> p c u", u=1)
                                    .broadcast_to([128, kb, 128]),
                                op=AL.is_equal)
                            gv = gfs[:, 0:kb*TW2].rearrange("p (k t) -> p k t", t=TW2)
                            z = e2b.tile([128, KBLK * 1024], dt.bfloat16, tag="z")
                            nc.vector.tensor_tensor(
                                out=z[:, 0:kb*1024].rearrange("p (k t) -> p k t", t=1024),
                                in0=gv[:, :, 0:1024],
                                in1=gfd[:, 0:kb*1024].rearrange("p (k t) -> p k t", t=1024),
                                op=AL.add)
                            nc.scalar.activation(out=z[:, 0:kb*1024], in_=z[:, 0:kb*1024],
                                                 func=AF.Abs)
                            zv = z[:, 0:kb*1024].rearrange("p (k t) -> p k t", t=1024)
                            eab = e2b.tile([128, 4 * KBLK], dt.float32, tag="eab")
                            ranges2 = ((0, 0, P2[0]), (1, P2[0], 512),
                                       (2, 512, 512 + P2[1]), (3, 512 + P2[1], 1024))
                            for (g, lo, hi) in ranges2:
                                if lo == hi:
                                    nc.vector.memset(eab[:, g:4*kb:4], 0.0)
                                else:
                                    nc.vector.tensor_reduce(
                                        out=eab[:, g:4*kb:4], in_=zv[:, :, lo:hi],
                                        axis=mybir.AxisListType.X, op=AL.add)
                            e2t = e2b.tile([128, 2 * KBLK], dt.float32, tag="e2t")
                            nc.vector.tensor_tensor(
                                out=e2t[:, 0:2*kb], in0=eab[:, 0:4*kb:2],
                                in1=eab[:, 1:4*kb:2], op=AL.subtract)
                            nc.vector.tensor_tensor(
                                out=e2t[:, 0:2*kb].rearrange("p (k h) -> p k h", h=2),
                                in0=e2t[:, 0:2*kb].rearrange("p (k h) -> p k h", h=2),
                                in1=gv[:, :, 1024:1026], op=AL.add)
                            ex = e2b.tile([128, 2 * KBLK], dt.float32, tag="ex")
                            nc.scalar.activation(out=ex[:, 0:2*kb], in_=e2t[:, 0:2*kb],
                                                 func=AF.Exp)
                            # denominator rhs: [1 | ex1/ex0] per chunk
                            dn = e2b.tile([128, 2 * KBLK], dt.bfloat16, tag="dn")
                            nc.vector.memset(dn[:, 0:2*kb], 1.0)
                            rr = e2b.tile([128, KBLK], dt.float32, tag="rr")
                            nc.vector.reciprocal(rr[:, 0:kb], ex[:, 0:2*kb:2])
                            nc.vector.tensor_tensor(out=dn[:, 1:2*kb:2], in0=rr[:, 0:kb],
                                                    in1=ex[:, 1:2*kb:2], op=AL.mult)
                            sS = e2a.tile([128, 2 * KBLK * 128], dt.bfloat16, tag="sS")
                            for h in range(2):
                                nc.vector.tensor_tensor(
                                    out=sS[:, h*KBLK*128:h*KBLK*128+kb*128]
                                        .rearrange("p (c o) -> p c o", o=128),
                                    in0=oh[:, 0:kb*128].rearrange("p (c o) -> p c o", o=128),
                                    in1=ex[:, h:2*kb:2].rearrange("p (c u) -> p c u", u=1)
                                        .broadcast_to([128, kb, 128]),
                                    op=AL.mult)
                            for c in range(kb):
                                st = first and (c == 0)
                                sp = last and (c == kb - 1)
                                nc.tensor.matmul(
                                    agg0[:, :], lhsT=sS[:, c*128:(c+1)*128],
                                    rhs=gfs[:, c*TW2:c*TW2+512], start=st, stop=sp)
                                nc.tensor.matmul(
                                    agg1[:, :], lhsT=sS[:, KBLK*128+c*128:KBLK*128+(c+1)*128],
                                    rhs=gfs[:, c*TW2+512:c*TW2+1024], start=st, stop=sp)
                                nc.tensor.matmul(
                                    den[:, :], lhsT=sS[:, c*128:(c+1)*128],
                                    rhs=dn[:, 2*c:2*c+2], start=st, stop=sp)
                        # ---- epilogue: h2 = sum_h relu(num/den + res) ----
                        dsb = e2b.tile([128, 2], dt.float32, tag="dsb")
                        nc.vector.tensor_scalar_add(dsb[:, :], den[:, :], 1e-20)
                        rcp = e2b.tile([128, 2], dt.float32, tag="rcp")
                        nc.vector.reciprocal(rcp[:, :], dsb[:, :])
                        # th_h = (agg_h * rcp_h) * invc2_h  (c2 prescale divided out)
                        th0 = e2b.tile([128, 512], dt.bfloat16, tag="th0")
                        th1 = e2b.tile([128, 512], dt.bfloat16, tag="th1")
                        nc.vector.scalar_tensor_tensor(
                            out=th0[:, :], in0=agg0[:, :], scalar=rcp[:, 0:1],
                            in1=ic2[:, 0:512], op0=AL.mult, op1=AL.mult)
                        nc.vector.scalar_tensor_tensor(
                            out=th1[:, :], in0=agg1[:, :], scalar=rcp[:, 1:2],
                            in1=ic2[:, 512:1024], op0=AL.mult, op1=AL.mult)
                        nc.vector.tensor_tensor(out=th0[:, :], in0=th0[:, :],
                                                in1=rs2[:, w*1024:w*1024+512], op=AL.add)
                        nc.vector.tensor_tensor(out=th1[:, :], in0=th1[:, :],
                                                in1=rs2[:, w*1024+512:(w+1)*1024], op=AL.add)
                        nc.vector.tensor_scalar_max(th0[:, :], th0[:, :], 0.0)
                        nc.vector.tensor_scalar_max(th1[:, :], th1[:, :], 0.0)
                        # ---- final projection: relu(sum_h relu(t_h)) == sum_h
                        # relu(t_h), so project each head separately (with its
                        # own perm2-ordered Wp copy) and accumulate in PSUM.
                        pf = e2pf.tile([128, OUT], dt.float32, space="PSUM", tag="pf")
                        h2T = e2b.tile([128, 1024], dt.bfloat16, tag="h2T")
                        for h, th in ((0, th0), (1, th1)):
                            for j in range(4):
                                jj = h * 4 + j
                                tpv = pf[:, jj*64:(jj+1)*64].bitcast(dt.bfloat16)
                                nc.tensor.transpose(out=tpv, in_=th[:, j*128:(j+1)*128],
                                                    identity=ident[:, :])
                                nc.vector.tensor_copy(out=h2T[:, jj*128:(jj+1)*128],
                                                      in_=tpv)
                        for k in range(8):
                            nc.tensor.matmul(pf[:, :], lhsT=h2T[:, k*128:(k+1)*128],
                                             rhs=wp[:, k*OUT:(k+1)*OUT],
                                             start=(k == 0), stop=(k == 7 and not has_bias))
                        if has_bias:
                            nc.tensor.matmul(pf[:, :], lhsT=ebias[:, :], rhs=bpt[:, :],
                                             start=False, stop=True)
                        of = e2b.tile([128, OUT], dt.float32, tag="of")
                        nc.vector.tensor_copy(out=of[:, :], in_=pf[:, :])
                        rows = min(128, NLOC - w * 128)
                        nc.sync.dma_start(out=out_d[w*128:w*128+rows, :],
                                          in_=of[:rows, :])
                        cbase += mw

    nc.compile()
    return nc


def kernel(**inputs) -> np.ndarray:
    import sys
    if "/opt/trn_rl_repo" not in sys.path:
        sys.path.insert(0, "/opt/trn_rl_repo")
    from concourse.bass_utils import run_bass_kernel_spmd

    in_maps, Mw, n_chunks, meta = _host_prep(**inputs)
    key = ("prog", tuple(Mw), tuple(sorted(meta.items())))
    if key not in _CACHE:
        _CACHE[key] = _build_program(Mw, n_chunks, meta)
    nc = _CACHE[key]
    res = run_bass_kernel_spmd(nc, in_maps, core_ids=list(range(NCORES)))
    return np.concatenate([res.results[m]["out"] for m in range(NCORES)], axis=0)



# revision 29
# speedup vs baseline: 1.1833x; 1.0350x over previous
"""Two-layer GATv2 (DGL GATv2Conv x2 + projection) on 8 Trainium2 NeuronCores.

Sharding: nodes partitioned across 8 cores (1250 each); edges assigned to the
owner of dst; weights replicated; src features exchanged via AllGather of the
per-layer gather table (bf16).

Math: lrelu(z) = 0.6 z + 0.4 |z| (slope 0.2), so the attention logit
e = sum_d a_d lrelu(z_d) = 0.6(as_u + ad_v) + 0.4 sum_d a_d |z_d| with
as = x @ (W_s @ a), ad = x @ (W_d @ a) carried as extra table columns.
Softmax is unnormalized: numerator and denominator accumulate in the same PSUM
window via matmuls with an exp-scaled one-hot scatter matrix; division happens
per 128-node window in the epilogue.

Edge phase: dma_gather fetches up to 1024 edge rows per instruction (src rows
from the allgathered table, dst rows from the local fd table); all elementwise,
activation and reduce work is batched over whole gather blocks.

conv1 table row (640 cols): [fs_h0(256) | 1 | fs_h1(256) | 1 | a0 a1 | 0pad]
  -> per-head agg matmul rhs [fs_h | 1] is contiguous (fused denominator).
conv2 table row (1152 cols): [fs_h0(512) | fs_h1(512) | a0 a1 | 0pad]
  -> denominators via one matmul per chunk with rhs [1 | ex1/ex0].
"""
import numpy as np

N, E = 10000, 160000
IN, HID, OUT, H = 1024, 512, 512, 2
D1 = HID // H
D2 = HID
NCORES = 8
NLOC = N // NCORES
WIN = 128
NW = (NLOC + WIN - 1) // WIN
KBLK = 8

TW1 = 640
TW2 = 1152

_CACHE = {}


def _bf16(x):
    import ml_dtypes
    return np.asarray(x, dtype=np.float32).astype(ml_dtypes.bfloat16)


def _pack_idx16(flat):
    n = len(flat)
    a = np.zeros((16, n // 16), np.int16)
    a[np.arange(n) % 16, np.arange(n) // 16] = flat
    return np.tile(a, (8, 1))


def _host_prep(x, src, dst, W1s, b1s, W1d, b1d, attn1, W1r, b1r,
               W2s, b2s, W2d, b2d, attn2, W2r, b2r, Wp, bp):
    src = np.asarray(src).astype(np.int64)
    dst = np.asarray(dst).astype(np.int64)
    x = np.asarray(x, dtype=np.float32)

    core_of = dst // NLOC
    wloc = (dst % NLOC) // WIN
    e_lists = [[np.nonzero((core_of == m) & (wloc == w))[0] for w in range(NW)]
               for m in range(NCORES)]
    Mw = [max(1, max((len(e_lists[m][w]) + 127) // 128 for m in range(NCORES)))
          for w in range(NW)]
    n_chunks = int(sum(Mw))

    src_idx = np.zeros((NCORES, n_chunks * 128), np.int64)
    dst_idx = np.zeros((NCORES, n_chunks * 128), np.int64)
    # dst position within the 128-node window, per edge; 255 marks padding
    # (is_equal against column iota 0..127 then yields an all-zero row).
    dpos = np.zeros((NCORES, 128, n_chunks), np.float32)
    for m in range(NCORES):
        ci = 0
        for w in range(NW):
            el = e_lists[m][w]
            el = el[np.argsort(src[el], kind="stable")]  # DRAM row locality
            ne = len(el)
            npad = Mw[w] * 128
            s_pad = np.zeros(npad, np.int64)
            d_pad = np.zeros(npad, np.int64)
            v_pad = np.full(npad, 255, np.int64)
            sr = src[el]
            sm, srr = sr // NLOC, sr % NLOC
            sj = srr // 640
            s_pad[:ne] = sj * (NCORES * 640) + sm * 640 + (srr - sj * 640)
            d_pad[:ne] = dst[el] - m * NLOC
            v_pad[:ne] = dst[el] - m * NLOC - w * WIN
            src_idx[m, ci*128:(ci+Mw[w])*128] = s_pad
            dst_idx[m, ci*128:(ci+Mw[w])*128] = d_pad
            dpos[m, :, ci:ci+Mw[w]] = v_pad.reshape(Mw[w], 128).T
            ci += Mw[w]

    def mk_alpha(W, b, attn, d):
        ac = np.stack([W[:, h*d:(h+1)*d] @ attn[h] for h in range(H)], axis=1) * 0.6
        ab = np.array([0.6 * attn[h] @ b[h*d:(h+1)*d] for h in range(H)], np.float32)
        return ac.astype(np.float32), ab

    attn1 = np.asarray(attn1, np.float32); attn2 = np.asarray(attn2, np.float32)
    W1s = np.asarray(W1s, np.float32); W1d = np.asarray(W1d, np.float32)
    W1r = np.asarray(W1r, np.float32); W2r = np.asarray(W2r, np.float32)
    W2s = np.asarray(W2s, np.float32); W2d = np.asarray(W2d, np.float32)
    b1s = np.asarray(b1s, np.float32); b1d = np.asarray(b1d, np.float32)
    b1r = np.asarray(b1r, np.float32); b2r = np.asarray(b2r, np.float32)
    b2s = np.asarray(b2s, np.float32); b2d = np.asarray(b2d, np.float32)
    a1s, a1s_b = mk_alpha(W1s, b1s, attn1, D1)
    a1d, a1d_b = mk_alpha(W1d, b1d, attn1, D1)

    # Per-head sign-split permutation: columns with a_d >= 0 first, then
    # negative; every fs/fd table column is pre-scaled by c = 0.4|a_d| so the
    # logit reduce is a plain (pos-sum minus neg-sum) with no multiply pass.
    # conv1: h1 comes out permuted AND scaled by c; absorbed into W2* input
    # rows (host-side divide). conv2: the c-scale is divided out on-device in
    # the epilogue (fused into the existing rcp multiply), and the per-head
    # permutation rides into per-head copies of Wp.
    def sign_split(attn, d):
        perms, counts, scales = [], [], []
        for h in range(H):
            a = attn[h]
            neg = a < 0
            perm = np.argsort(neg, kind="stable")
            perms.append(perm)
            counts.append(int((~neg).sum()))
            scales.append(np.maximum(0.4 * np.abs(a[perm]), 1e-12).astype(np.float32))
        return perms, tuple(counts), scales

    perm1, P1, c1 = sign_split(attn1, D1)
    perm2, P2, c2 = sign_split(attn2, D2)

    def permscale_cols(W, perms, scales, d):
        # W [*, H*d] -> per-head column permutation and scale
        out = np.empty_like(W)
        for h in range(H):
            out[..., h*d:(h+1)*d] = W[..., h*d:(h+1)*d][..., perms[h]] * scales[h]
        return out

    W1s_t = permscale_cols(W1s, perm1, c1, D1)
    W1d_t = permscale_cols(W1d, perm1, c1, D1)
    W1r_t = permscale_cols(W1r, perm1, c1, D1)
    b1s_t = permscale_cols(b1s, perm1, c1, D1)
    b1d_t = permscale_cols(b1d, perm1, c1, D1)
    b1r_t = permscale_cols(b1r, perm1, c1, D1)

    # W2* consume h1' = c1-scaled, perm1-permuted h1: rows permuted, divided.
    def absorb_rows(W):
        out = np.empty_like(W)
        for h in range(H):
            blk = W[h*D1 + np.asarray(perm1[h])] / c1[h][:, None]
            out[h*D1:(h+1)*D1] = blk
        return out

    W2s_a = absorb_rows(W2s); W2d_a = absorb_rows(W2d); W2r_a = absorb_rows(W2r)
    # conv2 extras consume h1' (conv1-transformed), so build them from the
    # absorbed weights.
    a2s, a2s_b = mk_alpha(W2s_a, b2s, attn2, D2)
    a2d, a2d_b = mk_alpha(W2d_a, b2d, attn2, D2)
    W2s_t = permscale_cols(W2s_a, perm2, c2, D2)
    W2d_t = permscale_cols(W2d_a, perm2, c2, D2)
    b2s_t = permscale_cols(b2s, perm2, c2, D2)
    b2d_t = permscale_cols(b2d, perm2, c2, D2)
    # conv2 residual: permuted but NOT scaled (the c2 divide happens on-device
    # before the residual add).
    def perm_cols(W, perms, d):
        out = np.empty_like(W)
        for h in range(H):
            out[..., h*d:(h+1)*d] = W[..., h*d:(h+1)*d][..., perms[h]]
        return out

    W2r_t = perm_cols(W2r_a, perm2, D2)
    b2r_t = perm_cols(b2r, perm2, D2)

    # conv1 T block: [fs0 | ones | fs1 | ones | a0 a1 | pad]
    def blk1(W, alpha):
        B = np.zeros((IN, TW1), np.float32)
        B[:, 0:256] = W[:, 0:256]
        B[:, 257:513] = W[:, 256:512]
        B[:, 514:516] = alpha
        return B

    def brow1(b, ab, with_ones):
        r = np.zeros(TW1, np.float32)
        r[0:256] = b[0:256]; r[257:513] = b[256:512]; r[514:516] = ab
        if with_ones:
            r[256] = 1.0; r[513] = 1.0
        return r

    W1cat = np.concatenate([blk1(W1s_t, a1s), blk1(W1d_t, a1d), W1r_t], axis=1)
    b1cat = np.zeros((128, W1cat.shape[1]), np.float32)
    b1cat[0, 0:TW1] = brow1(b1s_t, a1s_b, True)
    b1cat[0, TW1:2*TW1] = brow1(b1d_t, a1d_b, False)
    b1cat[0, 2*TW1:] = b1r_t

    # conv2 T block: [fs0 | fs1 | a0 a1 | pad]
    def blk2(W, alpha):
        B = np.zeros((HID, TW2), np.float32)
        B[:, 0:1024] = W
        B[:, 1024:1026] = alpha
        return B

    W2cat = np.concatenate([blk2(W2s_t, a2s), blk2(W2d_t, a2d), W2r_t], axis=1)
    b2cat = np.zeros((128, W2cat.shape[1]), np.float32)
    b2cat[0, 0:1024] = b2s_t; b2cat[0, 1024:1026] = a2s_b
    b2cat[0, TW2:TW2+1024] = b2d_t; b2cat[0, TW2+1024:TW2+1026] = a2d_b
    b2cat[0, 2*TW2:] = b2r_t

    # per-head Wp with conv2's head permutation on its input rows
    Wp = np.asarray(Wp, np.float32)
    Wp2 = np.concatenate([Wp[np.asarray(perm2[h])] for h in range(H)], axis=0)

    invc2 = np.concatenate([1.0 / c2[h] for h in range(H)])
    invc2_t = np.tile(invc2.reshape(1, -1), (128, 1))

    bpcat = np.zeros((128, OUT), np.float32)
    bpcat[0, :] = np.asarray(bp, np.float32)
    has_bias = bool(max(float(np.abs(np.asarray(b, np.float32)).max()) for b in
                        (b1s, b1d, b1r, b2s, b2d, b2r, bp)) > 0)

    ident = np.eye(128, dtype=np.float32)
    ebias = np.zeros((128, 128), np.float32); ebias[0, :] = 1.0
    iotac = np.tile(np.arange(128, dtype=np.float32), (128, 1))

    shared = {
        "w1cat": _bf16(W1cat), "b1cat": _bf16(b1cat),
        "w2cat": _bf16(W2cat), "b2cat": _bf16(b2cat),
        "wp": _bf16(Wp2), "bpcat": _bf16(bpcat),
        "invc2": _bf16(invc2_t),
        "ident": _bf16(ident), "ebias": _bf16(ebias),
        "iotac": _bf16(iotac),
    }
    in_maps = []
    for m in range(NCORES):
        xm = x[m*NLOC:(m+1)*NLOC]
        xT = np.zeros((IN, 1280), np.float32)
        xT[:, :NLOC] = xm.T
        im = dict(shared)
        im["xt"] = _bf16(xT)
        im["sidx"] = _pack_idx16(src_idx[m])
        im["didx"] = _pack_idx16(dst_idx[m])
        im["dpos"] = _bf16(dpos[m])
        in_maps.append(im)
    meta = {"has_bias": has_bias, "P1": P1, "P2": P2}
    return in_maps, Mw, n_chunks, meta


def _blocks(mw):
    out, c = [], 0
    while c < mw:
        k = min(KBLK, mw - c)
        out.append((c, k))
        c += k
    return out


def _build_program(Mw, n_chunks, meta=None, repeat=1):
    has_bias = meta["has_bias"]
    P1 = meta["P1"]
    P2 = meta["P2"]
    import sys
    if "/opt/trn_rl_repo" not in sys.path:
        sys.path.insert(0, "/opt/trn_rl_repo")
    import concourse.bass as bass
    import concourse.bacc as bacc
    import concourse.mybir as mybir
    import concourse.tile as tile

    dt = mybir.dt
    AF = mybir.ActivationFunctionType
    AL = mybir.AluOpType

    nc = bacc.Bacc("TRN2", target_bir_lowering=False, debug=False,
                   num_devices=NCORES)

    W1W = 2 * TW1 + 512    # 1792
    W2W = 2 * TW2 + 1024   # 3328
    RG = [list(range(NCORES))]

    xt_d = nc.dram_tensor("xt", [IN, 1280], dt.bfloat16, kind="ExternalInput")
    w1_d = nc.dram_tensor("w1cat", [IN, W1W], dt.bfloat16, kind="ExternalInput")
    b1_d = nc.dram_tensor("b1cat", [128, W1W], dt.bfloat16, kind="ExternalInput")
    w2_d = nc.dram_tensor("w2cat", [HID, W2W], dt.bfloat16, kind="ExternalInput")
    b2_d = nc.dram_tensor("b2cat", [128, W2W], dt.bfloat16, kind="ExternalInput")
    wp_d = nc.dram_tensor("wp", [2 * HID, OUT], dt.bfloat16, kind="ExternalInput")
    bp_d = nc.dram_tensor("bpcat", [128, OUT], dt.bfloat16, kind="ExternalInput")
    ic2_d = nc.dram_tensor("invc2", [128, 1024], dt.bfloat16, kind="ExternalInput")
    id_d = nc.dram_tensor("ident", [128, 128], dt.bfloat16, kind="ExternalInput")
    eb_d = nc.dram_tensor("ebias", [128, 128], dt.bfloat16, kind="ExternalInput")
    si_d = nc.dram_tensor("sidx", [128, n_chunks * 8], dt.int16, kind="ExternalInput")
    di_d = nc.dram_tensor("didx", [128, n_chunks * 8], dt.int16, kind="ExternalInput")
    dp_d = nc.dram_tensor("dpos", [128, n_chunks], dt.bfloat16, kind="ExternalInput")
    io_d = nc.dram_tensor("iotac", [128, 128], dt.bfloat16, kind="ExternalInput")

    NCH = (NLOC + 639) // 640
    t1_own = nc.dram_tensor("t1_own", [NCH * 640, TW1], dt.bfloat16, kind="Internal")
    t1_full = nc.dram_tensor("t1_full", [NCH * NCORES * 640, TW1], dt.bfloat16,
                             kind="Internal", addr_space="Shared")
    fd1q = nc.dram_tensor("fd1q", [NLOC, 512], dt.bfloat16, kind="Internal")
    t2_own = nc.dram_tensor("t2_own", [NCH * 640, TW2], dt.bfloat16, kind="Internal")
    t2_full = nc.dram_tensor("t2_full", [NCH * NCORES * 640, TW2], dt.bfloat16,
                             kind="Internal", addr_space="Shared")
    fd2q = nc.dram_tensor("fd2q", [NLOC, 1024], dt.bfloat16, kind="Internal")
    out_d = nc.dram_tensor("out", [NLOC, OUT], dt.float32, kind="ExternalOutput")

    def mm_cols(ncols):
        splits, c = [], 0
        while c < ncols:
            n_ = min(512, ncols - c)
            splits.append((c, c + n_))
            c += n_
        return splits

    def node_matmul(ps, lhs_of_k, nk, w_sb, ww, b_sb, ones_cols, ebias):
        """Accumulate sum_k lhsT_k.T @ W_k into ps[:, 0:ww] (+ bias row)."""
        spans = mm_cols(ww)
        for k in range(nk):
            lhs = lhs_of_k(k)
            for si, (c0, c1) in enumerate(spans):
                last = (k == nk - 1)
                need_bias = has_bias or any(c0 <= oc < c1 for oc in ones_cols)
                nc.tensor.matmul(ps[:, c0:c1], lhsT=lhs, rhs=w_sb[:, k*ww+c0:k*ww+c1],
                                 start=(k == 0), stop=(last and not need_bias))
        for (c0, c1) in spans:
            need_bias = has_bias or any(c0 <= oc < c1 for oc in ones_cols)
            if need_bias:
                nc.tensor.matmul(ps[:, c0:c1], lhsT=ebias[:, :], rhs=b_sb[:, c0:c1],
                                 start=False, stop=True)

    with tile.TileContext(nc) as tc:
        with tc.tile_pool(name="cst", bufs=1) as cst, \
             tc.tile_pool(name="res", bufs=1) as resid:

            ident = cst.tile([128, 128], dt.bfloat16)
            nc.sync.dma_start(out=ident[:, :], in_=id_d[:, :])
            ebias = cst.tile([128, 128], dt.bfloat16)
            nc.sync.dma_start(out=ebias[:, :], in_=eb_d[:, :])
            ic2 = cst.tile([128, 1024], dt.bfloat16)
            nc.sync.dma_start(out=ic2[:, :], in_=ic2_d[:, :])
            sidx = cst.tile([128, n_chunks * 8], dt.int16)
            nc.sync.dma_start(out=sidx[:, :], in_=si_d[:, :])
            didx = cst.tile([128, n_chunks * 8], dt.int16)
            nc.sync.dma_start(out=didx[:, :], in_=di_d[:, :])
            dpos = cst.tile([128, n_chunks], dt.bfloat16)
            nc.sync.dma_start(out=dpos[:, :], in_=dp_d[:, :])
            iot = cst.tile([128, 128], dt.bfloat16)
            nc.sync.dma_start(out=iot[:, :], in_=io_d[:, :])

            rs1 = resid.tile([128, NW * 512], dt.bfloat16)
            h1 = resid.tile([128, NW * 512], dt.bfloat16)
            rs2 = resid.tile([128, NW * 1024], dt.bfloat16)

            for rep in range(repeat):
                # ================= conv1 node matmuls =================
                with tc.tile_pool(name="n1", bufs=2) as n1p, \
                     tc.tile_pool(name="n1w", bufs=1) as n1w, \
                     tc.tile_pool(name="n1ps", bufs=1, space="PSUM") as n1ps:
                    xt_sb = n1w.tile([128, 8 * 1280], dt.bfloat16)
                    for k in range(8):
                        nc.sync.dma_start(out=xt_sb[:, k*1280:(k+1)*1280],
                                          in_=xt_d[k*128:(k+1)*128, :])
                    w1 = n1w.tile([128, 8 * W1W], dt.bfloat16)
                    for k in range(8):
                        nc.sync.dma_start(out=w1[:, k*W1W:(k+1)*W1W],
                                          in_=w1_d[k*128:(k+1)*128, :])
                    b1 = n1w.tile([128, W1W], dt.bfloat16)
                    nc.sync.dma_start(out=b1[:, :], in_=b1_d[:, :])
                    for nt in range(NW):
                        ps = n1ps.tile([128, W1W], dt.float32, space="PSUM")
                        node_matmul(ps,
                                    lambda k: xt_sb[:, k*1280 + nt*128: k*1280 + (nt+1)*128],
                                    8, w1, W1W, b1, (256, 513), ebias)
                        t1t = n1p.tile([128, TW1], dt.bfloat16, tag="t1t")
                        nc.vector.tensor_copy(out=t1t[:, :], in_=ps[:, 0:TW1])
                        fdt = n1p.tile([128, 512], dt.bfloat16, tag="fdt")
                        nc.vector.tensor_copy(
                            out=fdt[:, :].rearrange("p (h d) -> p h d", d=256),
                            in_=ps[:, TW1:TW1+514].rearrange("p (h d) -> p h d", d=257)
                                [:, :, 0:256])
                        nc.vector.tensor_copy(out=rs1[:, nt*512:(nt+1)*512],
                                              in_=ps[:, 2*TW1:W1W])
                        rows = min(128, NLOC - nt * 128)
                        nc.sync.dma_start(out=t1_own[nt*128:nt*128+rows, :],
                                          in_=t1t[:rows, :])
                        nc.sync.dma_start(out=fd1q[nt*128:nt*128+rows, :],
                                          in_=fdt[:rows, :])
                        if (nt + 1) * 128 % 640 == 0 or nt == NW - 1:
                            j = nt // 5
                            nc.gpsimd.collective_compute(
                                "AllGather", mybir.AluOpType.bypass, replica_groups=RG,
                                ins=[t1_own[j*640:(j+1)*640, :]],
                                outs=[t1_full[j*NCORES*640:(j+1)*NCORES*640, :]])

                # ================= conv1 edge phase =================
                with tc.tile_pool(name="e1a", bufs=3) as e1a, \
                     tc.tile_pool(name="e1b", bufs=3) as e1b, \
                     tc.tile_pool(name="e1agg", bufs=2, space="PSUM") as e1agg:
                    cbase = 0
                    for w in range(NW):
                        mw = Mw[w]
                        agg0 = e1agg.tile([128, 257], dt.float32, space="PSUM", tag="agg0")
                        agg1 = e1agg.tile([128, 257], dt.float32, space="PSUM", tag="agg1")
                        for bi, (c0, kb) in enumerate(_blocks(mw)):
                            ci = cbase + c0
                            first = (bi == 0)
                            last = (c0 + kb == mw)
                            gfs = e1a.tile([128, KBLK * TW1], dt.bfloat16, tag="gfs")
                            nc.gpsimd.dma_gather(
                                out_ap=gfs[:, 0:kb*TW1].rearrange("p (k t) -> p k t", t=TW1),
                                in_ap=t1_full[:, :], idxs_ap=sidx[:, ci*8:(ci+kb)*8],
                                num_idxs=kb*128, num_idxs_reg=kb*128, elem_size=TW1)
                            gfd = e1b.tile([128, KBLK * 512], dt.bfloat16, tag="gfd")
                            nc.gpsimd.dma_gather(
                                out_ap=gfd[:, 0:kb*512].rearrange("p (k t) -> p k t", t=512),
                                in_ap=fd1q[:, :], idxs_ap=didx[:, ci*8:(ci+kb)*8],
                                num_idxs=kb*128, num_idxs_reg=kb*128, elem_size=512)
                            oh = e1a.tile([128, KBLK * 128], dt.bfloat16, tag="oh")
                            nc.vector.tensor_tensor(
                                out=oh[:, 0:kb*128].rearrange("p (c o) -> p c o", o=128),
                                in0=iot[:, :].rearrange("p (u o) -> p u o", u=1)
                                    .broadcast_to([128, kb, 128]),
                                in1=dpos[:, ci:ci+kb].rearrange("p (c u) -> p c u", u=1)
                                    .broadcast_to([128, kb, 128]),
                                op=AL.is_equal)
                            gv = gfs[:, 0:kb*TW1].rearrange("p (k t) -> p k t", t=TW1)
                            z = e1b.tile([128, KBLK * 512], dt.bfloat16, tag="z")
                            nc.vector.tensor_tensor(
                                out=z[:, 0:kb*512].rearrange("p (k h d) -> p k h d", h=2, d=256),
                                in0=gv[:, :, 0:514].rearrange("p k (h d) -> p k h d", d=257)
                                    [:, :, :, 0:256],
                                in1=gfd[:, 0:kb*512].rearrange("p (k h d) -> p k h d", h=2, d=256),
                                op=AL.add)
                            nc.scalar.activation(out=z[:, 0:kb*512], in_=z[:, 0:kb*512],
                                                 func=AF.Abs)
                            zv = z[:, 0:kb*512].rearrange("p (k t) -> p k t", t=512)
                            # eab4 col layout: 4k + 2h + s (s: 0=pos, 1=neg)
                            eab = e1b.tile([128, 4 * KBLK], dt.float32, tag="eab")
                            ranges1 = ((0, 0, P1[0]), (1, P1[0], 256),
                                       (2, 256, 256 + P1[1]), (3, 256 + P1[1], 512))
                            for (g, lo, hi) in ranges1:
                                if lo == hi:
                                    nc.vector.memset(eab[:, g:4*kb:4], 0.0)
                                else:
                                    nc.vector.tensor_reduce(
                                        out=eab[:, g:4*kb:4], in_=zv[:, :, lo:hi],
                                        axis=mybir.AxisListType.X, op=AL.add)
                            # extras: only as_u — the ad_v term is constant per
                            # dst node and cancels in the edge softmax.
                            e2 = e1b.tile([128, 2 * KBLK], dt.float32, tag="e2")
                            nc.vector.tensor_tensor(
                                out=e2[:, 0:2*kb], in0=eab[:, 0:4*kb:2],
                                in1=eab[:, 1:4*kb:2], op=AL.subtract)
                            nc.vector.tensor_tensor(
                                out=e2[:, 0:2*kb].rearrange("p (k h) -> p k h", h=2),
                                in0=e2[:, 0:2*kb].rearrange("p (k h) -> p k h", h=2),
                                in1=gv[:, :, 514:516], op=AL.add)
                            ex = e1b.tile([128, 2 * KBLK], dt.float32, tag="ex")
                            nc.scalar.activation(out=ex[:, 0:2*kb], in_=e2[:, 0:2*kb],
                                                 func=AF.Exp)
                            sS = e1a.tile([128, 2 * KBLK * 128], dt.bfloat16, tag="sS")
                            for h in range(2):
                                nc.vector.tensor_tensor(
                                    out=sS[:, h*KBLK*128:h*KBLK*128+kb*128]
                                        .rearrange("p (c o) -> p c o", o=128),
                                    in0=oh[:, 0:kb*128].rearrange("p (c o) -> p c o", o=128),
                                    in1=ex[:, h:2*kb:2].rearrange("p (c u) -> p c u", u=1)
                                        .broadcast_to([128, kb, 128]),
                                    op=AL.mult)
                            for c in range(kb):
                                st = first and (c == 0)
                                sp = last and (c == kb - 1)
                                nc.tensor.matmul(
                                    agg0[:, :], lhsT=sS[:, c*128:(c+1)*128],
                                    rhs=gfs[:, c*TW1:c*TW1+257], start=st, stop=sp)
                                nc.tensor.matmul(
                                    agg1[:, :], lhsT=sS[:, KBLK*128+c*128:KBLK*128+(c+1)*128],
                                    rhs=gfs[:, c*TW1+257:c*TW1+514], start=st, stop=sp)
                        # ---- window epilogue ----
                        dsb = e1b.tile([128, 2], dt.float32, tag="dsb")
                        nc.vector.tensor_copy(out=dsb[:, 0:1], in_=agg0[:, 256:257])
                        nc.vector.tensor_copy(out=dsb[:, 1:2], in_=agg1[:, 256:257])
                        nc.vector.tensor_scalar_add(dsb[:, :], dsb[:, :], 1e-20)
                        rcp = e1b.tile([128, 2], dt.float32, tag="rcp")
                        nc.vector.reciprocal(rcp[:, :], dsb[:, :])
                        hw = h1[:, w*512:(w+1)*512]
                        nc.vector.tensor_scalar_mul(hw[:, 0:256], agg0[:, 0:256], rcp[:, 0:1])
                        nc.vector.tensor_scalar_mul(hw[:, 256:512], agg1[:, 0:256], rcp[:, 1:2])
                        nc.vector.tensor_tensor(out=hw[:, :], in0=hw[:, :],
                                                in1=rs1[:, w*512:(w+1)*512], op=AL.add)
                        nc.vector.tensor_scalar_max(hw[:, :], hw[:, :], 0.0)
                        cbase += mw

                # ================= conv2 node matmuls =================
                with tc.tile_pool(name="n2", bufs=2) as n2p, \
                     tc.tile_pool(name="n2w", bufs=1) as n2w, \
                     tc.tile_pool(name="n2ps", bufs=1, space="PSUM") as n2ps:
                    w2 = n2w.tile([128, 4 * W2W], dt.bfloat16)
                    for k in range(4):
                        nc.sync.dma_start(out=w2[:, k*W2W:(k+1)*W2W],
                                          in_=w2_d[k*128:(k+1)*128, :])
                    b2 = n2w.tile([128, W2W], dt.bfloat16)
                    nc.sync.dma_start(out=b2[:, :], in_=b2_d[:, :])
                    HALF = 1792   # bank-aligned split of 3328 (1792 + 1536)
                    for nt in range(NW):
                        tp = n2ps.tile([128, 128], dt.bfloat16, space="PSUM", tag="tp")
                        hT = n2p.tile([128, 512], dt.bfloat16, tag="hT")
                        for j in range(4):
                            nc.tensor.transpose(out=tp[:, :],
                                                in_=h1[:, nt*512+j*128:nt*512+(j+1)*128],
                                                identity=ident[:, :])
                            nc.vector.tensor_copy(out=hT[:, j*128:(j+1)*128], in_=tp[:, :])
                        t2t = n2p.tile([128, TW2], dt.bfloat16, tag="t2t")
                        fdt = n2p.tile([128, 1024], dt.bfloat16, tag="fdt2")
                        ps = n2ps.tile([128, HALF], dt.float32, space="PSUM")
                        for hf, (base, hw_) in enumerate(((0, HALF), (HALF, W2W - HALF))):
                            spans = mm_cols(hw_)
                            for k in range(4):
                                for (c0, c1) in spans:
                                    nc.tensor.matmul(
                                        ps[:, c0:c1], lhsT=hT[:, k*128:(k+1)*128],
                                        rhs=w2[:, k*W2W+base+c0:k*W2W+base+c1],
                                        start=(k == 0),
                                        stop=(k == 3 and not has_bias))
                            if has_bias:
                                for (c0, c1) in spans:
                                    nc.tensor.matmul(ps[:, c0:c1], lhsT=ebias[:, :],
                                                     rhs=b2[:, base+c0:base+c1],
                                                     start=False, stop=True)
                            seg = [(0, TW2, "t2"), (TW2, 2*TW2, "fd"), (2*TW2, W2W, "rs")]
                            for (s0, s1, kind) in seg:
                                lo, hi = max(s0, base), min(s1, base + hw_)
                                if lo >= hi:
                                    continue
                                srcv = ps[:, lo-base:hi-base]
                                if kind == "t2":
                                    nc.vector.tensor_copy(out=t2t[:, lo:hi], in_=srcv)
                                elif kind == "fd":
                                    if lo - TW2 < 1024:
                                        nc.vector.tensor_copy(
                                            out=fdt[:, lo-TW2:min(hi-TW2, 1024)],
                                            in_=srcv[:, 0:min(hi, TW2+1024)-lo])
                                else:
                                    nc.vector.tensor_copy(
                                        out=rs2[:, nt*1024+lo-2*TW2:nt*1024+hi-2*TW2],
                                        in_=srcv)
                        rows = min(128, NLOC - nt * 128)
                        nc.sync.dma_start(out=t2_own[nt*128:nt*128+rows, :],
                                          in_=t2t[:rows, :])
                        nc.sync.dma_start(out=fd2q[nt*128:nt*128+rows, :],
                                          in_=fdt[:rows, :])
                        if (nt + 1) * 128 % 640 == 0 or nt == NW - 1:
                            j = nt // 5
                            nc.gpsimd.collective_compute(
                                "AllGather", mybir.AluOpType.bypass, replica_groups=RG,
                                ins=[t2_own[j*640:(j+1)*640, :]],
                                outs=[t2_full[j*NCORES*640:(j+1)*NCORES*640, :]])

                # ================= conv2 edge phase + final =================
                with tc.tile_pool(name="e2a", bufs=2) as e2a, \
                     tc.tile_pool(name="e2b", bufs=2) as e2b, \
                     tc.tile_pool(name="e2w", bufs=1) as e2w, \
                     tc.tile_pool(name="e2agg", bufs=2, space="PSUM") as e2agg, \
                     tc.tile_pool(name="e2pf", bufs=1, space="PSUM") as e2pf:
                    wp = e2w.tile([128, 8 * OUT], dt.bfloat16)
                    for k in range(8):
                        nc.sync.dma_start(out=wp[:, k*OUT:(k+1)*OUT],
                                          in_=wp_d[k*128:(k+1)*128, :])
                    bpt = e2w.tile([128, OUT], dt.bfloat16)
                    nc.sync.dma_start(out=bpt[:, :], in_=bp_d[:, :])
                    cbase = 0
                    for w in range(NW):
                        mw = Mw[w]
                        agg0 = e2agg.tile([128, 512], dt.float32, space="PSUM", tag="agg0")
                        agg1 = e2agg.tile([128, 512], dt.float32, space="PSUM", tag="agg1")
                        den = e2agg.tile([128, 2], dt.float32, space="PSUM", tag="den")
                        for bi, (c0, kb) in enumerate(_blocks(mw)):
                            ci = cbase + c0
                            first = (bi == 0)
                            last = (c0 + kb == mw)
                            gfs = e2a.tile([128, KBLK * TW2], dt.bfloat16, tag="gfs")
                            nc.gpsimd.dma_gather(
                                out_ap=gfs[:, 0:kb*TW2].rearrange("p (k t) -> p k t", t=TW2),
                                in_ap=t2_full[:, :], idxs_ap=sidx[:, ci*8:(ci+kb)*8],
                                num_idxs=kb*128, num_idxs_reg=kb*128, elem_size=TW2)
                            gfd = e2b.tile([128, KBLK * 1024], dt.bfloat16, tag="gfd")
                            nc.gpsimd.dma_gather(
                                out_ap=gfd[:, 0:kb*1024].rearrange("p (k t) -> p k t", t=1024),
                                in_ap=fd2q[:, :], idxs_ap=didx[:, ci*8:(ci+kb)*8],
                                num_idxs=kb*128, num_idxs_reg=kb*128, elem_size=1024)
                            oh = e2a.tile([128, KBLK * 128], dt.bfloat16, tag="oh")
                            nc.vector.tensor_tensor(
                                out=oh[:, 0:kb*128].rearrange("p (c o) -> p c o", o=128),
                                in0=iot[:, :].rearrange("p (u o) -> p u o", u=1)
                                    .broadcast_to([128, kb, 128]),
                                in1=dpos[:, ci:ci+kb].rearrange("p (c u) -> p c u", u=1)
                                    .broadcast_to([128, kb, 128]),
                                op=AL.is_equal)
                            gv = gfs[:, 0:kb*TW2].rearrange("p (k t) -> p k t", t=TW2)
                            z = e2b.tile([128, KBLK * 1024], dt.bfloat16, tag="z")
                            nc.vector.tensor_tensor(
                                out=z[:, 0:kb*1024].rearrange("p (k t) -> p k t", t=1024),
                                in0=gv[:, :, 0:1024],
                                in1=gfd[:, 0:kb*1024].rearrange("p (k t) -> p k t", t=1024),
                                op=AL.add)
                            nc.scalar.activation(out=z[:, 0:kb*1024], in_=z[:, 0:kb*1024],
                                                 func=AF.Abs)
                            zv = z[:, 0:kb*1024].rearrange("p (k t) -> p k t", t=1024)
                            eab = e2b.tile([128, 4 * KBLK], dt.float32, tag="eab")
                            ranges2 = ((0, 0, P2[0]), (1, P2[0], 512),
                                       (2, 512, 512 + P2[1]), (3, 512 + P2[1], 1024))
                            for (g, lo, hi) in ranges2:
                                if lo == hi:
                                    nc.vector.memset(eab[:, g:4*kb:4], 0.0)
                                else:
                                    nc.vector.tensor_reduce(
                                        out=eab[:, g:4*kb:4], in_=zv[:, :, lo:hi],
                                        axis=mybir.AxisListType.X, op=AL.add)
                            e2t = e2b.tile([128, 2 * KBLK], dt.float32, tag="e2t")
                            nc.vector.tensor_tensor(
                                out=e2t[:, 0:2*kb], in0=eab[:, 0:4*kb:2],
                                in1=eab[:, 1:4*kb:2], op=AL.subtract)
                            nc.vector.tensor_tensor(
                                out=e2t[:, 0:2*kb].rearrange("p (k h) -> p k h", h=2),
                                in0=e2t[:, 0:2*kb].rearrange("p (k h) -> p k h", h=2),
                                in1=gv[:, :, 1024:1026], op=AL.add)
                            ex = e2b.tile([128, 2 * KBLK], dt.float32, tag="ex")
                            nc.scalar.activation(out=ex[:, 0:2*kb], in_=e2t[:, 0:2*kb],
                                                 func=AF.Exp)
                            # denominator rhs: [1 | ex1/ex0] per chunk
                            dn = e2b.tile([128, 2 * KBLK], dt.bfloat16, tag="dn")
                            nc.vector.memset(dn[:, 0:2*kb], 1.0)
                            rr = e2b.tile([128, KBLK], dt.float32, tag="rr")
                            nc.vector.reciprocal(rr[:, 0:kb], ex[:, 0:2*kb:2])
                            nc.vector.tensor_tensor(out=dn[:, 1:2*kb:2], in0=rr[:, 0:kb],
                                                    in1=ex[:, 1:2*kb:2], op=AL.mult)
                            sS = e2a.tile([128, 2 * KBLK * 128], dt.bfloat16, tag="sS")
                            for h in range(2):
                                nc.vector.tensor_tensor(
                                    out=sS[:, h*KBLK*128:h*KBLK*128+kb*128]
                                        .rearrange("p (c o) -> p c o", o=128),
                                    in0=oh[:, 0:kb*128].rearrange("p (c o) -> p c o", o=128),
                                    in1=ex[:, h:2*kb:2].rearrange("p (c u) -> p c u", u=1)
                                        .broadcast_to([128, kb, 128]),
                                    op=AL.mult)
                            for c in range(kb):
                                st = first and (c == 0)
                                sp = last and (c == kb - 1)
                                nc.tensor.matmul(
                                    agg0[:, :], lhsT=sS[:, c*128:(c+1)*128],
                                    rhs=gfs[:, c*TW2:c*TW2+512], start=st, stop=sp)
                                nc.tensor.matmul(
                                    agg1[:, :], lhsT=sS[:, KBLK*128+c*128:KBLK*128+(c+1)*128],
                                    rhs=gfs[:, c*TW2+512:c*TW2+1024], start=st, stop=sp)
                                nc.tensor.matmul(
                                    den[:, :], lhsT=sS[:, c*128:(c+1)*128],
                                    rhs=dn[:, 2*c:2*c+2], start=st, stop=sp)
                        # ---- epilogue: h2 = sum_h relu(num/den + res) ----
                        dsb = e2b.tile([128, 2], dt.float32, tag="dsb")
                        nc.vector.tensor_scalar_add(dsb[:, :], den[:, :], 1e-20)
                        rcp = e2b.tile([128, 2], dt.float32, tag="rcp")
                        nc.vector.reciprocal(rcp[:, :], dsb[:, :])
                        # th_h = (agg_h * rcp_h) * invc2_h  (c2 prescale divided out)
                        th0 = e2b.tile([128, 512], dt.bfloat16, tag="th0")
                        th1 = e2b.tile([128, 512], dt.bfloat16, tag="th1")
                        nc.vector.scalar_tensor_tensor(
                            out=th0[:, :], in0=agg0[:, :], scalar=rcp[:, 0:1],
                            in1=ic2[:, 0:512], op0=AL.mult, op1=AL.mult)
                        nc.vector.scalar_tensor_tensor(
                            out=th1[:, :], in0=agg1[:, :], scalar=rcp[:, 1:2],
                            in1=ic2[:, 512:1024], op0=AL.mult, op1=AL.mult)
                        nc.vector.tensor_tensor(out=th0[:, :], in0=th0[:, :],
                                                in1=rs2[:, w*1024:w*1024+512], op=AL.add)
                        nc.vector.tensor_tensor(out=th1[:, :], in0=th1[:, :],
                                                in1=rs2[:, w*1024+512:(w+1)*1024], op=AL.add)
                        nc.vector.tensor_scalar_max(th0[:, :], th0[:, :], 0.0)
                        nc.vector.tensor_scalar_max(th1[:, :], th1[:, :], 0.0)
                        # ---- final projection: relu(sum_h relu(t_h)) == sum_h
                        # relu(t_h), so project each head separately (with its
                        # own perm2-ordered Wp copy) and accumulate in PSUM.
                        pf = e2pf.tile([128, OUT], dt.float32, space="PSUM", tag="pf")
                        h2T = e2b.tile([128, 1024], dt.bfloat16, tag="h2T")
                        for h, th in ((0, th0), (1, th1)):
                            for j in range(4):
                                jj = h * 4 + j
                                tpv = pf[:, jj*64:(jj+1)*64].bitcast(dt.bfloat16)
                                nc.tensor.transpose(out=tpv, in_=th[:, j*128:(j+1)*128],
                                                    identity=ident[:, :])
                                nc.vector.tensor_copy(out=h2T[:, jj*128:(jj+1)*128],
                                                      in_=tpv)
                        for k in range(8):
                            nc.tensor.matmul(pf[:, :], lhsT=h2T[:, k*128:(k+1)*128],
                                             rhs=wp[:, k*OUT:(k+1)*OUT],
                                             start=(k == 0), stop=(k == 7 and not has_bias))
                        if has_bias:
                            nc.tensor.matmul(pf[:, :], lhsT=ebias[:, :], rhs=bpt[:, :],
                                             start=False, stop=True)
                        of = e2b.tile([128, OUT], dt.float32, tag="of")
                        nc.vector.tensor_copy(out=of[:, :], in_=pf[:, :])
                        rows = min(128, NLOC - w * 128)
                        nc.sync.dma_start(out=out_d[w*128:w*128+rows, :],
                                          in_=of[:rows, :])
                        cbase += mw

    nc.compile()
    return nc


def kernel(**inputs) -> np.ndarray:
    import sys
    if "/opt/trn_rl_repo" not in sys.path:
        sys.path.insert(0, "/opt/trn_rl_repo")
    from concourse.bass_utils import run_bass_kernel_spmd

    in_maps, Mw, n_chunks, meta = _host_prep(**inputs)
    key = ("prog", tuple(Mw), tuple(sorted(meta.items())))
    if key not in _CACHE:
        _CACHE[key] = _build_program(Mw, n_chunks, meta)
    nc = _CACHE[key]
    res = run_bass_kernel_spmd(nc, in_maps, core_ids=list(range(NCORES)))
    return np.concatenate([res.results[m]["out"] for m in range(NCORES)], axis=0)

